# revision 9
# baseline (speedup 1.0000x reference)
"""Trainium2 Bass kernel for Mesh2GridDecoder (GraphCast-style mesh->grid
message passing + output MLP), distributed over 8 NeuronCores.

Strategy: grid nodes are sharded across cores by 128-row destination blocks
(blocks load-balanced across cores by edge count so the scatter-sum is
core-local); mesh node features and all weights are replicated.  Everything
runs in bf16 with fp32 PSUM accumulation.

Math restructuring (exact, up to float re-association):
  h     = silu(attrs @ emb_w0 + emb_b0)                       per edge
  pre2  = src@Ws + dst@Wd + e_emb@We + edge_b0
        = mesh_proj[src] + grid_proj[dst] + h @ W_he
    with mesh_proj = mesh@Ws, grid_proj = grid@Wd + (emb_b1@We + edge_b0),
         W_he = emb_w1 @ We
  hid2  = silu(pre2)
  agg   = (S@h)@emb_w1 + (S@hid2)@edge_w1 + cnt (x) (emb_b1+edge_b1)
  pre3  = grid@W0a + (S@h)@U1 + (S@hid2)@U2 + cnt (x) v3 + node_b0
  hid3  = silu(pre3)
  pre4  = grid@out_w0 + hid3@V + b4
  out   = silu(pre4) @ out_w1 + out_b1

Edges are sorted by dst and grouped into 128-row dst blocks.  Per 128-edge
chunk a 0/1 selector S_c[e,d] = (dst_in_block[e]==d) drives the scatter-sum
on the tensor engine; its transpose S_cT[d,e] turns the dst-feature gather
grid_proj[dst] into a matmul accumulated straight into the pre2 PSUM.
mesh/grid arrive pre-transposed from the host so no on-device transposes are
needed outside the aggregation read-back.
"""
import math
import numpy as np
import ml_dtypes

import concourse.bass as bass
import concourse.tile as tile
from concourse import mybir
from concourse import bass_utils
from concourse import library_config
from concourse.vector_clock import ScopedClock

BF16 = mybir.dt.bfloat16
F32 = mybir.dt.float32
I16 = mybir.dt.int16
AF = mybir.ActivationFunctionType
ALU = mybir.AluOpType
bf = ml_dtypes.bfloat16

N_MESH = 10242
N_GRID = 65160
N_EDGES = 195480
D = 512
OUTD = 471
NCORES = 8
NM = 10368                      # padded mesh rows (81 chunks of 128)
NGB = (N_GRID + 127) // 128     # 510 global 128-row dst blocks
NBLK = 64                       # dst blocks per core (8*64 = 512 slots)
NGS = NBLK * 128                # 8192 local grid rows per core
SPLIT_WAITS = True              # walrus 1-wait/inst workaround (off for CoreSim)


# ---------------------------------------------------------------- tile patch
def _patched_drain_and_barrier(self, tick_clock, wait_clock):
    # This walrus build accepts at most 1 sync wait per instruction; the
    # stock tail drain carries one wait per active proc.  Emit explicit
    # wait_ge instructions instead.
    probe = self.nc.sync.nop()
    if probe.ins.sync_info is None:
        probe.ins.sync_info = mybir.SyncInfo(on_wait=[], on_update=[])
    wait_clock.add_sem_waits(probe.ins, ScopedClock({None: tick_clock.global_clock}))
    waits = list(probe.ins.sync_info.on_wait)
    del probe.ins.sync_info.on_wait[:]
    name2sem = {s.name: s for s in self.sems.allocated().values()}
    for w in waits:
        self.nc.sync.wait_ge(name2sem[w.ant_name], w.wait_value)
    self.nc.sync.drain()
    self.nc.all_engine_barrier()
    assert self.sems is not None
    popped = self.nc._tile_sem_poison_stack.pop()
    assert popped is self._sem_poison
    self.nc.clear_and_free_semaphores(list(self.sems.allocated().values()))
    self.nc.all_engine_barrier()


tile.TileContext._drain_and_barrier = _patched_drain_and_barrier


# ------------------------------------------------------------------- helpers
def _wrap_idx(idx: np.ndarray) -> np.ndarray:
    """dma_gather index layout: index i at [i % 16, i // 16], the 16-row
    block replicated down all 128 partitions."""
    assert idx.size % 16 == 0
    w = idx.astype(np.int16).reshape(-1, 16).T  # [16, n/16]
    return np.ascontiguousarray(np.tile(w, (8, 1)))


def _cdiv(a, b):
    return (a + b - 1) // b


def _pad128(n):
    return _cdiv(n, 128) * 128


def build_bass(CAPS):
    """Build the per-core Bass program.  CAPS[b] = 128-edge chunks in dst
    block b (same for every core)."""
    CAPS = list(CAPS)
    NCHUNK = sum(CAPS)
    ECP = NCHUNK * 128
    NCP = _pad128(NCHUNK)          # padded chunk-count (dstb_col width)
    IW = _pad128(ECP // 16)        # padded srcidx width

    # blob row offsets (must match _prep's packing order)
    offs = {}
    r = 0

    def takerows(name, nrows):
        nonlocal r
        offs[name] = r
        r += nrows

    takerows("meshT", NM * 4)            # [512, NM] -> (f*81+b, j) rows
    takerows("gridT", NGS * 4)           # [512, NGS] -> ((k*128+p)*64 + c) rows
    for w in ("ws", "wd", "whe", "u1", "u2", "w0a", "ow0", "v", "ow1"):
        takerows(w, 2048)                # [512, 512] -> ((k*128+p)*4+q, j)
    takerows("emb0", 20)                 # [5, 512]
    takerows("v3b3", 8)                  # [2, 512]
    takerows("b2", 4)                    # [1, 512]
    takerows("b4", 4)
    takerows("ob1", 4)
    takerows("iota128", 128)             # [128, 128]
    takerows("iotacol", 128)             # [128, 128] (col 0 = arange)
    takerows("cntones", 128)             # [2, 8192]
    takerows("attrs5", 5 * ECP // 128)   # [5, ECP]
    takerows("dstrep", ECP)              # [128, ECP]
    takerows("dstcol", NCP)              # [128, NCP]
    RBLOB = r

    nc = bass.Bass("TRN2", target_bir_lowering=False, debug=False,
                   num_devices=NCORES)

    blob = nc.dram_tensor("blob", [RBLOB, 128], BF16, kind="ExternalInput").ap()
    idx16 = nc.dram_tensor("idx16", [IW + 512, 128], I16,
                           kind="ExternalInput").ap()
    outt = nc.dram_tensor("outt", [NGS, 512], BF16,
                          kind="ExternalOutput").ap()

    def rows(name, n):
        o = offs[name]
        return blob[o:o + n, :]

    meshT_v = rows("meshT", NM * 4).rearrange("(k p b) j -> p k b j",
                                              p=128, b=81)
    gridT_v = rows("gridT", NGS * 4).rearrange("(k p c q) j -> p k c q j",
                                               p=128, c=16, q=4)
    w_views = {w: rows(w, 2048).rearrange("(k p q) j -> p k q j", p=128, q=4)
               for w in ("ws", "wd", "whe", "u1", "u2", "w0a", "ow0", "v",
                         "ow1")}
    emb0_v = rows("emb0", 20).rearrange("(a q) j -> a q j", a=5)
    v3b3_v = rows("v3b3", 8).rearrange("(a q) j -> a q j", a=2)
    b2_v = rows("b2", 4).rearrange("(a q) j -> a q j", a=1)
    b4_v = rows("b4", 4).rearrange("(a q) j -> a q j", a=1)
    ob1_v = rows("ob1", 4).rearrange("(a q) j -> a q j", a=1)
    iota_v = rows("iota128", 128)
    iotacol_v = rows("iotacol", 128)
    cnt_v = rows("cntones", 128).rearrange("(a c q) j -> a c q j", a=2, c=16)
    attrs_v = rows("attrs5", 5 * ECP // 128).rearrange(
        "(a c) j -> a c j", a=5)
    dstrep_v = rows("dstrep", ECP).rearrange("(p c) j -> p c j", p=128)
    dstcol_v = rows("dstcol", NCP).rearrange("(p c) j -> p c j", p=128)
    srci_v = idx16[:IW, :].rearrange("(p c) j -> p c j", p=128)
    iong_v = idx16[IW:IW + 512, :].rearrange("(p c) j -> p c j", p=128)

    with tile.TileContext(nc) as tc:
        with tc.tile_pool(name="const", bufs=1) as cp, \
             tc.tile_pool(name="dram", bufs=1, space="DRAM") as dp, \
             tc.tile_pool(name="io", bufs=3) as io, \
             tc.tile_pool(name="io2", bufs=2) as io2, \
             tc.tile_pool(name="wk", bufs=3) as wk, \
             tc.tile_pool(name="psA", bufs=4, space="PSUM") as psA, \
             tc.tile_pool(name="psAgg", bufs=2, space="PSUM") as psAgg:

            nc.gpsimd.load_library(library_config.mlp)
            regs = {}

            def reg_of(n):
                if n not in regs:
                    regs[n] = nc.gpsimd.to_reg(n)
                return regs[n]

            # ---- DRAM scratch
            meshproj = dp.tile([NM, D], BF16)
            aggH = dp.tile([NGS, D], BF16)
            aggI = dp.tile([NGS, D], BF16)

            # ---- resident constants in SBUF
            def cload(view, shape, dt, tag):
                t = cp.tile(shape, dt, tag=tag)
                nc.sync.dma_start(t[:], view)
                return t

            w_sb = {w: cload(w_views[w], [128, 4, 4, 128], BF16, w)
                    for w in w_views}
            emb0_sb = cload(emb0_v, [5, 4, 128], BF16, "emb0")
            v3b3_sb = cload(v3b3_v, [2, 4, 128], BF16, "v3b3")
            b2_sb = cload(b2_v, [1, 4, 128], BF16, "b2")
            b4_sb = cload(b4_v, [1, 4, 128], BF16, "b4")
            ob1_sb = cload(ob1_v, [1, 4, 128], BF16, "ob1")
            iota_sb = cload(iota_v, [128, 128], BF16, "iota")
            iotacol_b = cload(iotacol_v, [128, 128], BF16, "iotacol")
            iotacol_sb = cp.tile([128, 1], F32, tag="iotacolf")
            nc.vector.tensor_copy(iotacol_sb[:], iotacol_b[:, 0:1])
            dstcol_b = cload(dstcol_v, [128, NCP // 128, 128], BF16, "dstcol")
            dstcol_sb = cp.tile([128, NCP // 128, 128], F32, tag="dstcolf")
            nc.vector.tensor_copy(dstcol_sb[:], dstcol_b[:])
            srci_sb = cp.tile([128, IW], I16, tag="srci")
            for g in range(IW // 128):
                nc.sync.dma_start(srci_sb[:, g * 128:(g + 1) * 128],
                                  srci_v[:, g, :])
            iong_sb = cload(iong_v, [128, 4, 128], I16, "iong")
            ones1_sb = cp.tile([1, 128], BF16, tag="ones1")
            nc.vector.memset(ones1_sb[:], 1.0)
            onesrow_sb = cp.tile([1, 512], BF16, tag="onesrow")
            nc.vector.memset(onesrow_sb[:], 1.0)
            gridproj_sb = cp.tile([128, NBLK, D], BF16, tag="gproj")

            # ---- P1: mesh_proj = mesh @ Ws  (row-major bf16 -> DRAM)
            for mb in range(NM // 128):
                mT = io.tile([128, 4, 128], BF16, tag="p1g")
                nc.sync.dma_start(mT[:], meshT_v[:, :, mb, :])
                ps = psA.tile([128, D], F32, tag="mm")
                for k in range(4):
                    nc.tensor.matmul(ps[:], mT[:, k, :], w_sb["ws"][:, k],
                                     start=(k == 0), stop=(k == 3))
                mp = io.tile([128, D], BF16, tag="p1o")
                nc.scalar.copy(mp[:], ps[:])
                nc.scalar.dma_start(meshproj[mb * 128:(mb + 1) * 128, :],
                                    mp[:])

            # ---- P2: grid_proj = grid @ Wd + b2  (SBUF resident)
            for gb in range(NBLK):
                gT = io.tile([128, 4, 128], BF16, tag="p2g")
                nc.sync.dma_start(gT[:], gridT_v[:, :, gb // 4, gb % 4, :])
                ps = psA.tile([128, D], F32, tag="mm")
                for k in range(4):
                    nc.tensor.matmul(ps[:], gT[:, k, :], w_sb["wd"][:, k],
                                     start=(k == 0), stop=False)
                nc.tensor.matmul(ps[:], ones1_sb[:], b2_sb[:, :, :],
                                 start=False, stop=True)
                nc.vector.tensor_copy(gridproj_sb[:, gb, :], ps[:])

            # ---- P3: edge phase
            CMAX = max(CAPS)
            coff = 0
            for b in range(NBLK):
                CAP = CAPS[b]
                L = CAP * 128
                e0 = coff * 128
                attrs_sb = io.tile([5, CMAX, 128], BF16, tag="attrs")
                nc.sync.dma_start(attrs_sb[:, :CAP, :],
                                  attrs_v[:, coff:coff + CAP, :])
                dstrep_sb = io.tile([128, CMAX, 128], BF16, tag="dstrep")
                nc.sync.dma_start(dstrep_sb[:, :CAP, :],
                                  dstrep_v[:, coff:coff + CAP, :])
                srcGt = io2.tile([128, CMAX, D], BF16, tag="srcG")
                srcG = srcGt[:, :CAP, :]
                nc.gpsimd.dma_gather(
                    srcG, meshproj[:],
                    srci_sb[:, e0 // 16:(e0 + L) // 16],
                    num_idxs=L, num_idxs_reg=reg_of(L), elem_size=D)

                aggH_ps = psAgg.tile([128, D], F32, tag="aggH")
                aggI_ps = psAgg.tile([128, D], F32, tag="aggI")

                for c in range(CAP):
                    cc = coff + c
                    a_c = attrs_sb[:, c, :]
                    # h edge-major
                    psz = psA.tile([128, D], F32, tag="mm")
                    nc.tensor.matmul(psz[:], a_c, emb0_sb[:],
                                     start=True, stop=True)
                    hR = wk.tile([128, D], BF16, tag="hR")
                    nc.scalar.activation(hR[:], psz[:], AF.Silu)
                    # h feature-major (4 small matmuls, K=5)
                    psF = psA.tile([128, D], F32, tag="mm")
                    for f in range(4):
                        nc.tensor.matmul(psF[:, f * 128:(f + 1) * 128],
                                         emb0_sb[:, f, :], a_c,
                                         start=True, stop=True)
                    hFa = wk.tile([128, D], BF16, tag="hFa")
                    nc.scalar.activation(hFa[:], psF[:], AF.Silu)
                    # selector S_cT[d, e] = (d == dst_in_block[e])
                    S_cT = wk.tile([128, 128], BF16, tag="S_cT")
                    nc.vector.tensor_scalar(
                        S_cT[:], dstrep_sb[:, c, :], iotacol_sb[:, 0:1],
                        None, op0=ALU.is_equal)
                    # pre2 = h @ W_he + grid_proj[dst] (+b2 folded in)
                    ps2 = psA.tile([128, D], F32, tag="mm")
                    for f in range(4):
                        nc.tensor.matmul(ps2[:], hFa[:, f * 128:(f + 1) * 128],
                                         w_sb["whe"][:, f],
                                         start=(f == 0), stop=False)
                    nc.tensor.matmul(ps2[:], S_cT[:], gridproj_sb[:, b, :],
                                     start=False, stop=True)
                    t2 = wk.tile([128, D], BF16, tag="t2")
                    nc.vector.tensor_tensor(t2[:], srcG[:, c, :], ps2[:],
                                            op=ALU.add)
                    hid2 = wk.tile([128, D], BF16, tag="hid2")
                    nc.scalar.activation(hid2[:], t2[:], AF.Silu)
                    # selector S_c[e, d] = (dst_in_block[e] == d)
                    S_c = wk.tile([128, 128], BF16, tag="S_c")
                    nc.vector.tensor_scalar(
                        S_c[:], iota_sb[:],
                        dstcol_sb[:, cc // 128, cc % 128:cc % 128 + 1],
                        None, op0=ALU.is_equal)
                    # scatter-sum into block agg tiles
                    nc.tensor.matmul(aggH_ps[:], S_c[:], hR[:],
                                     start=(c == 0), stop=(c == CAP - 1),
                                     skip_group_check=True)
                    nc.tensor.matmul(aggI_ps[:], S_c[:], hid2[:],
                                     start=(c == 0), stop=(c == CAP - 1),
                                     skip_group_check=True)

                aH = io.tile([128, D], BF16, tag="aH")
                nc.vector.tensor_copy(aH[:], aggH_ps[:])
                nc.scalar.dma_start(aggH[b * 128:(b + 1) * 128, :], aH[:])
                aI = io.tile([128, D], BF16, tag="aI")
                nc.vector.tensor_copy(aI[:], aggI_ps[:])
                nc.scalar.dma_start(aggI[b * 128:(b + 1) * 128, :], aI[:])
                coff += CAP

            # ---- P4: node + output MLPs, 512-row blocks
            for rb in range(NGS // 512):
                r0 = rb * 512
                cnt_sb = io.tile([2, 4, 128], BF16, tag="cnt")
                nc.sync.dma_start(cnt_sb[:], cnt_v[:, rb, :, :])
                gT = io2.tile([128, 4, 4, 128], BF16, tag="gT4")
                nc.sync.dma_start(gT[:], gridT_v[:, :, rb, :, :])
                isl = iong_sb[:, rb // 4, (rb % 4) * 32:(rb % 4) * 32 + 32]
                aHT = io2.tile([128, 4, D], BF16, tag="aHT")
                nc.gpsimd.dma_gather(aHT[:], aggH[:], isl,
                                     num_idxs=512, num_idxs_reg=reg_of(512),
                                     elem_size=D, transpose=True)
                aIT = io2.tile([128, 4, D], BF16, tag="aIT")
                nc.gpsimd.dma_gather(aIT[:], aggI[:], isl,
                                     num_idxs=512, num_idxs_reg=reg_of(512),
                                     elem_size=D, transpose=True)

                h3 = io2.tile([128, 4, 512], BF16, tag="h3")
                for g in range(4):
                    gs = slice(g * 128, (g + 1) * 128)
                    ps3 = psA.tile([128, 512], F32, tag="mm")
                    for k in range(4):
                        nc.tensor.matmul(ps3[:], w_sb["w0a"][:, k, g, :],
                                         gT[:, k], start=(k == 0), stop=False)
                    for k in range(4):
                        nc.tensor.matmul(ps3[:], w_sb["u1"][:, k, g, :],
                                         aHT[:, k, :], start=False, stop=False)
                    for k in range(4):
                        nc.tensor.matmul(ps3[:], w_sb["u2"][:, k, g, :],
                                         aIT[:, k, :], start=False, stop=False)
                    nc.tensor.matmul(ps3[:], v3b3_sb[:, g, :], cnt_sb[:],
                                     start=False, stop=True)
                    nc.scalar.activation(h3[:, g, :], ps3[:], AF.Silu)

                h4 = io2.tile([128, 4, 512], BF16, tag="h4")
                for g in range(4):
                    ps4 = psA.tile([128, 512], F32, tag="mm")
                    for k in range(4):
                        nc.tensor.matmul(ps4[:], w_sb["ow0"][:, k, g, :],
                                         gT[:, k], start=(k == 0), stop=False)
                    for k in range(4):
                        nc.tensor.matmul(ps4[:], w_sb["v"][:, k, g, :],
                                         h3[:, k, :], start=False, stop=False)
                    nc.tensor.matmul(ps4[:], b4_sb[:, g, :], onesrow_sb[:],
                                     start=False, stop=True)
                    nc.scalar.activation(h4[:, g, :], ps4[:], AF.Silu)

                for sc in range(4):
                    rs = slice(sc * 128, (sc + 1) * 128)
                    pso = psA.tile([128, 512], F32, tag="mm")
                    for k in range(4):
                        nc.tensor.matmul(pso[:], h4[:, k, rs],
                                         w_sb["ow1"][:, k],
                                         start=(k == 0), stop=False)
                    nc.tensor.matmul(pso[:], ones1_sb[:], ob1_sb[:],
                                     start=False, stop=True)
                    ot = io.tile([128, 512], BF16, tag="ot")
                    nc.vector.tensor_copy(ot[:], pso[:])
                    nc.scalar.dma_start(
                        outt[r0 + sc * 128:r0 + (sc + 1) * 128, :], ot[:])

    from concourse.library_overlay import lower_extended_insts
    lower_extended_insts(nc)   # fill .instr of InstISA subclasses (load_library)
    if SPLIT_WAITS:
        _split_multi_waits(nc)
    return nc


def _split_multi_waits(nc):
    """This walrus build allows at most ONE sync wait per instruction.
    Move surplus waits onto EventSemaphore carrier instructions inserted
    immediately before, on the same engine (semantically identical: the
    sequencer blocks on each in order)."""
    for f in nc.m.functions:
        for bb in f.blocks:
            insts = list(bb.instructions)
            if not any(i.sync_info is not None and len(i.sync_info.on_wait) > 1
                       for i in insts):
                continue
            new = []
            for ins in insts:
                si = ins.sync_info
                if si is not None and len(si.on_wait) > 1:
                    waits = list(si.on_wait)
                    for w in waits[:-1]:
                        c = mybir.InstEventSemaphore(
                            name=f"I-w{nc.next_id()}", engine=ins.engine,
                            ins=[], outs=[],
                            sync_info=mybir.SyncInfo(on_wait=[w], on_update=[]))
                        new.append(c)
                    del si.on_wait[:]
                    si.on_wait.append(waits[-1])
                new.append(ins)
            bb.instructions = new


# ------------------------------------------------------------ host pipeline
def _prep(inputs):
    """Host-side fold/sort/pack. Returns (in_maps, CAPS, perm)."""
    mesh_f = np.asarray(inputs["mesh_node_features"])[0]   # [N_MESH, D]
    grid_f = np.asarray(inputs["grid_node_features"])[0]   # [N_GRID, D]
    attrs = np.asarray(inputs["edge_attrs"])               # [E, 4]
    esrc = np.asarray(inputs["edge_src"]).astype(np.int64)
    edst = np.asarray(inputs["edge_dst"]).astype(np.int64)

    # ---- fold weights (fp32 on host, cast bf16)
    W = {k: np.asarray(inputs[k], np.float32) for k in (
        "emb_w0", "emb_b0", "emb_w1", "emb_b1", "edge_w0", "edge_b0",
        "edge_w1", "edge_b1", "node_w0", "node_b0", "node_w1", "node_b1",
        "out_w0", "out_b0", "out_w1", "out_b1")}
    Ws, Wd, We = W["edge_w0"][:D], W["edge_w0"][D:2 * D], W["edge_w0"][2 * D:]
    W0a, W0b = W["node_w0"][:D], W["node_w0"][D:]
    W_he = W["emb_w1"] @ We
    b2 = W["emb_b1"] @ We + W["edge_b0"]
    U1 = W["emb_w1"] @ W0b
    U2 = W["edge_w1"] @ W0b
    v3 = (W["emb_b1"] + W["edge_b1"]) @ W0b
    V = W["node_w1"] @ W["out_w0"]
    b4 = W["node_b1"] @ W["out_w0"] + W["out_b0"]
    emb_w0b = np.concatenate([W["emb_w0"], W["emb_b0"][None]], 0)  # [5, D]
    v3b3 = np.stack([v3, W["node_b0"]], 0)                          # [2, D]
    ow1p = np.zeros((D, 512), np.float32)
    ow1p[:, :OUTD] = W["out_w1"]
    ob1p = np.zeros(512, np.float32)
    ob1p[:OUTD] = W["out_b1"]

    # ---- sort edges by global dst block, balance blocks across cores
    order = np.argsort(edst, kind="stable")
    esrc, edst, attrs = esrc[order], edst[order], attrs[order]
    gblk = edst // 128                       # global 128-row dst block
    counts = np.bincount(gblk, minlength=NGB)
    blk_rank = np.argsort(counts)[::-1]      # descending edge count
    # slot assignment: rank r -> core r%8, slot r//8 (pad to 512 with -1)
    blocks = np.full(NCORES * NBLK, -1, np.int64)
    blocks[:NGB] = blk_rank
    core_of_rank = np.arange(NCORES * NBLK) % NCORES
    slot_of_rank = np.arange(NCORES * NBLK) // NCORES
    CAPS = np.zeros(NBLK, np.int64)
    for s in range(NBLK):
        gbs = blocks[s * NCORES:(s + 1) * NCORES]
        mx = max(int(counts[g]) if g >= 0 else 0 for g in gbs)
        CAPS[s] = max(1, _cdiv(mx, 128))
    CAPS = tuple(int(x) for x in CAPS)
    NCHUNK = sum(CAPS)
    ECP = NCHUNK * 128
    NCP = _pad128(NCHUNK)
    IW = _pad128(ECP // 16)
    chunk_off = np.concatenate([[0], np.cumsum(CAPS)])

    # per-core block table: core c slot s -> global block id (or -1)
    blk_of = np.full((NCORES, NBLK), -1, np.int64)
    for rnk in range(NCORES * NBLK):
        blk_of[core_of_rank[rnk], slot_of_rank[rnk]] = blocks[rnk]

    meshT = np.zeros((D, NM), np.float32)
    meshT[:, :N_MESH] = mesh_f.T
    iotaNG = _wrap_idx(np.arange(NGS))
    iota128 = np.tile(np.arange(128, dtype=np.float32)[None], (128, 1))
    iotacol = np.zeros((128, 128), np.float32)
    iotacol[:, 0] = np.arange(128)

    shared_parts = []

    def add(name, arr):
        shared_parts.append((name, np.asarray(arr, np.float32)))

    add("meshT", meshT.reshape(-1, 128))
    # gridT placeholder per core (inserted at same offset per core below)
    add("gridT", np.zeros((NGS * 4, 128), np.float32))
    for nm_, w_ in (("ws", Ws), ("wd", Wd), ("whe", W_he), ("u1", U1),
                    ("u2", U2), ("w0a", W0a), ("ow0", W["out_w0"]),
                    ("v", V), ("ow1", ow1p)):
        add(nm_, w_.reshape(-1, 128))
    add("emb0", emb_w0b.reshape(-1, 128))
    add("v3b3", v3b3.reshape(-1, 128))
    add("b2", b2.reshape(-1, 128))
    add("b4", b4.reshape(-1, 128))
    add("ob1", ob1p.reshape(-1, 128))
    add("iota128", iota128)
    add("iotacol", iotacol)
    add("cntones", np.zeros((128, 128), np.float32))   # per-core below
    add("attrs5", np.zeros((5 * ECP // 128, 128), np.float32))
    add("dstrep", np.zeros((ECP, 128), np.float32))
    add("dstcol", np.zeros((NCP, 128), np.float32))

    names = [n for n, _ in shared_parts]
    base_arrays = {n: a for n, a in shared_parts}
    per_core_names = {"gridT", "cntones", "attrs5", "dstrep", "dstcol"}

    in_maps = []
    perm = []            # (core, local_row0, global_row0, nrows)
    for core in range(NCORES):
        src_p = np.zeros(ECP, np.int64)
        dib_p = np.full(ECP, 999.0, np.float32)
        att_p = np.zeros((ECP, 4), np.float32)
        grid_b = np.zeros((NGS, D), np.float32)
        cnt = np.zeros(NGS, np.float32)
        for s in range(NBLK):
            gb = blk_of[core, s]
            if gb < 0:
                continue
            g0 = gb * 128
            g1 = min(g0 + 128, N_GRID)
            sz = g1 - g0
            grid_b[s * 128:s * 128 + sz] = grid_f[g0:g1]
            perm.append((core, s * 128, g0, sz))
            m = gblk == gb
            n = int(m.sum())
            o = chunk_off[s] * 128
            assert n <= CAPS[s] * 128
            src_p[o:o + n] = esrc[m]
            dloc = (edst[m] - g0).astype(np.float32)
            dib_p[o:o + n] = dloc
            att_p[o:o + n] = attrs[m]
            cnt[s * 128:s * 128 + sz] = np.bincount(
                dloc.astype(np.int64), minlength=128)[:sz]

        attrsT5 = np.concatenate(
            [att_p.T, np.ones((1, ECP), np.float32)], 0)       # [5, ECP]
        dstrep = np.tile(dib_p[None], (128, 1))                 # [128, ECP]
        dstcol = np.zeros((128, NCP), np.float32)
        dstcol[:, :NCHUNK] = np.ascontiguousarray(
            dib_p.reshape(-1, 128).T)
        cntones = np.zeros((2, NGS), np.float32)
        cntones[0] = cnt
        cntones[1] = 1.0

        parts = []
        for n in names:
            if n == "gridT":
                parts.append(grid_b.T.reshape(-1, 128))
            elif n == "cntones":
                parts.append(cntones.reshape(-1, 128))
            elif n == "attrs5":
                parts.append(attrsT5.reshape(-1, 128))
            elif n == "dstrep":
                parts.append(dstrep.reshape(-1, 128))
            elif n == "dstcol":
                parts.append(dstcol.reshape(-1, 128))
            else:
                parts.append(base_arrays[n])
        blob = np.concatenate([p.astype(bf) for p in parts], 0)

        srcw = np.zeros((128, IW), np.int16)
        srcw[:, :ECP // 16] = _wrap_idx(src_p)
        idx = np.concatenate([np.ascontiguousarray(srcw).reshape(-1, 128),
                              iotaNG.reshape(-1, 128)], 0).astype(np.int16)
        in_maps.append({"blob": np.ascontiguousarray(blob),
                        "idx16": np.ascontiguousarray(idx)})
    return in_maps, CAPS, perm


_CACHE = {}


class _Runner:
    """Persistent jitted SPMD executor (avoids re-jitting per call)."""

    def __init__(self, nc):
        import jax
        from jax.experimental.shard_map import shard_map
        from jax.sharding import Mesh, PartitionSpec
        from concourse import bass2jax

        bass2jax.install_neuronx_cc_hook()
        self.nc = nc
        part_name = (nc.partition_id_tensor.name
                     if nc.partition_id_tensor else None)
        in_names, out_names, out_avals, zero_outs = [], [], [], []
        for alloc in nc.m.functions[0].allocations:
            if not isinstance(alloc, mybir.MemoryLocationSet):
                continue
            name = alloc.memorylocations[0].name
            if alloc.kind == "ExternalInput":
                if name != part_name:
                    in_names.append(name)
            elif alloc.kind == "ExternalOutput":
                shape = tuple(alloc.tensor_shape)
                dtype = mybir.dt.np(alloc.dtype)
                out_names.append(name)
                out_avals.append(jax.core.ShapedArray(shape, dtype))
                zero_outs.append(np.zeros(shape, dtype))
        self.in_names = list(in_names)
        self.out_names = out_names
        self.out_shapes = [tuple(a.shape) for a in out_avals]
        all_names = in_names + out_names
        if part_name is not None:
            all_names = all_names + [part_name]

        def _body(*args):
            operands = list(args)
            if part_name is not None:
                operands.append(bass2jax.partition_id_tensor())
            outs = bass2jax._bass_exec_p.bind(
                *operands,
                out_avals=tuple(out_avals),
                in_names=tuple(all_names),
                out_names=tuple(out_names),
                lowering_input_output_aliases=(),
                sim_require_finite=True,
                sim_require_nnan=True,
                nc=nc,
            )
            return tuple(outs)

        devices = jax.devices()[:NCORES]
        mesh = Mesh(np.asarray(devices), ("core",))
        nin = len(self.in_names) + len(out_names)
        self.fn = jax.jit(shard_map(
            _body, mesh=mesh,
            in_specs=(PartitionSpec("core"),) * nin,
            out_specs=(PartitionSpec("core"),) * len(out_names),
            check_rep=False))
        self.zero_outs = zero_outs
        self.sharding = jax.sharding.NamedSharding(mesh, PartitionSpec("core"))
        self.mesh = mesh
        self._avals = out_avals
        self._jax = jax

    def put(self, in_maps):
        """Concatenate per-core inputs on axis 0, device_put pre-sharded."""
        arrs = []
        for name in self.in_names:
            arrs.append(np.concatenate([m[name] for m in in_maps], axis=0))
        for z in self.zero_outs:
            arrs.append(np.concatenate([z] * NCORES, axis=0))
        return [self._jax.device_put(a, self.sharding) for a in arrs]

    def run(self, arrs):
        return self.fn(*arrs)

    def get(self, outs):
        res = [np.asarray(o) for o in outs]
        per_core = []
        for c in range(NCORES):
            d = {}
            for i, name in enumerate(self.out_names):
                n0 = self.out_shapes[i][0]
                d[name] = res[i][c * n0:(c + 1) * n0]
            per_core.append(d)
        return per_core


def _get_runner(CAPS) -> _Runner:
    if CAPS not in _CACHE:
        _CACHE[CAPS] = _Runner(build_bass(CAPS))
    return _CACHE[CAPS]


def kernel(**inputs) -> np.ndarray:
    in_maps, CAPS, perm = _prep(inputs)
    r = _get_runner(CAPS)
    outs = r.run(r.put(in_maps))
    per_core = r.get(outs)
    out = np.zeros((N_GRID, OUTD), np.float32)
    for core, l0, g0, sz in perm:
        o = per_core[core]["outt"].reshape(NGS, 512)
        out[g0:g0 + sz] = o[l0:l0 + sz, :OUTD].astype(np.float32)
    return out[None]


# revision 29
# speedup vs baseline: 1.7278x; 1.7278x over previous
"""Trainium2 Bass kernel for Mesh2GridDecoder (GraphCast-style mesh->grid
message passing + output MLP), distributed over 8 NeuronCores.

Strategy: grid nodes are sharded across cores by 128-row destination blocks
(blocks load-balanced across cores by edge count so the scatter-sum is
core-local); mesh node features and all weights are replicated.  Everything
runs in bf16 with fp32 PSUM accumulation.

Math restructuring (exact, up to float re-association):
  h     = silu(attrs @ emb_w0 + emb_b0)                       per edge
  pre2  = src@Ws + dst@Wd + e_emb@We + edge_b0
        = mesh_proj[src] + grid_proj[dst] + h @ W_he
    with mesh_proj = mesh@Ws, grid_proj = grid@Wd + (emb_b1@We + edge_b0),
         W_he = emb_w1 @ We
  hid2  = silu(pre2)
  agg   = (S@h)@emb_w1 + (S@hid2)@edge_w1 + cnt (x) (emb_b1+edge_b1)
  pre3  = grid@W0a + (S@h)@U1 + (S@hid2)@U2 + cnt (x) v3 + node_b0
  hid3  = silu(pre3)
  pre4  = grid@out_w0 + hid3@V + b4
  out   = silu(pre4) @ out_w1 + out_b1

Edges are sorted by dst and grouped into 128-row dst blocks.  Per 128-edge
chunk a 0/1 selector S_c[e,d] = (dst_in_block[e]==d) drives the scatter-sum
on the tensor engine; its transpose S_cT[d,e] turns the dst-feature gather
grid_proj[dst] into a matmul accumulated straight into the pre2 PSUM.
mesh/grid arrive pre-transposed from the host so no on-device transposes are
needed outside the aggregation read-back.
"""
import math
import numpy as np
import ml_dtypes

import concourse.bass as bass
import concourse.tile as tile
from concourse import mybir
from concourse import bass_utils
from concourse import library_config
from concourse.vector_clock import ScopedClock

BF16 = mybir.dt.bfloat16
F32 = mybir.dt.float32
I16 = mybir.dt.int16
AF = mybir.ActivationFunctionType
ALU = mybir.AluOpType
bf = ml_dtypes.bfloat16

N_MESH = 10242
N_GRID = 65160
N_EDGES = 195480
D = 512
OUTD = 471
NCORES = 8
NM = 10368                      # padded mesh rows (81 chunks of 128)
NGB = (N_GRID + 127) // 128     # 510 global 128-row dst blocks
NBLK = 64                       # dst blocks per core (8*64 = 512 slots)
NGS = NBLK * 128                # 8192 local grid rows per core
SPLIT_WAITS = True              # walrus 1-wait/inst workaround (off for CoreSim)


# ---------------------------------------------------------------- tile patch
def _patched_drain_and_barrier(self, tick_clock, wait_clock):
    # This walrus build accepts at most 1 sync wait per instruction; the
    # stock tail drain carries one wait per active proc.  Emit explicit
    # wait_ge instructions instead.
    probe = self.nc.sync.nop()
    if probe.ins.sync_info is None:
        probe.ins.sync_info = mybir.SyncInfo(on_wait=[], on_update=[])
    wait_clock.add_sem_waits(probe.ins, ScopedClock({None: tick_clock.global_clock}))
    waits = list(probe.ins.sync_info.on_wait)
    del probe.ins.sync_info.on_wait[:]
    name2sem = {s.name: s for s in self.sems.allocated().values()}
    for w in waits:
        self.nc.sync.wait_ge(name2sem[w.ant_name], w.wait_value)
    self.nc.sync.drain()
    self.nc.all_engine_barrier()
    assert self.sems is not None
    popped = self.nc._tile_sem_poison_stack.pop()
    assert popped is self._sem_poison
    self.nc.clear_and_free_semaphores(list(self.sems.allocated().values()))
    self.nc.all_engine_barrier()


tile.TileContext._drain_and_barrier = _patched_drain_and_barrier


# ------------------------------------------------------------------- helpers
def _wrap_idx(idx: np.ndarray) -> np.ndarray:
    """dma_gather index layout: index i at [i % 16, i // 16], the 16-row
    block replicated down all 128 partitions."""
    assert idx.size % 16 == 0
    w = idx.astype(np.int16).reshape(-1, 16).T  # [16, n/16]
    return np.ascontiguousarray(np.tile(w, (8, 1)))


def _cdiv(a, b):
    return (a + b - 1) // b


def _pad128(n):
    return _cdiv(n, 128) * 128


def build_bass(CAPS):
    """Build the per-core Bass program.  CAPS[b] = 128-edge chunks in dst
    block b (same for every core)."""
    CAPS = list(CAPS)
    NCHUNK = sum(CAPS)
    ECP = NCHUNK * 128
    NCP = _pad128(NCHUNK)          # padded chunk-count (dstb_col width)
    IW = _pad128(ECP // 16)        # padded srcidx width

    # blob row offsets (must match _prep's packing order)
    offs = {}
    r = 0

    def takerows(name, nrows):
        nonlocal r
        offs[name] = r
        r += nrows

    takerows("meshT", NM * 4)            # [512, NM] -> (f*81+b, j) rows
    takerows("gridT", NGS * 4)           # [512, NGS] -> ((k*128+p)*64 + c) rows
    for w in ("ws", "wd", "whe", "u1", "u2", "w0a", "ow0", "v", "ow1"):
        takerows(w, 2048)                # [512, 512] -> ((k*128+p)*4+q, j)
    takerows("emb0", 20)                 # [5, 512]
    takerows("v3b3", 8)                  # [2, 512]
    takerows("b2", 4)                    # [1, 512]
    takerows("b4", 4)
    takerows("ob1", 4)
    takerows("iota128", 128)             # [128, 128]
    takerows("ident", 128)               # [128, 128] eye
    takerows("iotacol", 128)             # [128, 128] (col 0 = arange)
    takerows("cntones", 128)             # [2, 8192]
    takerows("attrs5", 5 * ECP // 128)   # [5, ECP]
    takerows("dstrep", ECP)              # [128, ECP]
    takerows("dstcol", NCP)              # [128, NCP]
    RBLOB = r

    nc = bass.Bass("TRN2", target_bir_lowering=False, debug=False,
                   num_devices=NCORES)

    blob = nc.dram_tensor("blob", [RBLOB, 128], BF16, kind="ExternalInput").ap()
    idx16 = nc.dram_tensor("idx16", [IW + 512, 128], I16,
                           kind="ExternalInput").ap()
    outt = nc.dram_tensor("outt", [NGS, OUTD], BF16,
                          kind="ExternalOutput").ap()

    def rows(name, n):
        o = offs[name]
        return blob[o:o + n, :]

    meshT_v = rows("meshT", NM * 4).rearrange("(k p b) j -> p k b j",
                                              p=128, b=81)
    gridT_v = rows("gridT", NGS * 4).rearrange("(k p c q) j -> p k c q j",
                                               p=128, c=16, q=4)
    w_views = {w: rows(w, 2048).rearrange("(k p q) j -> p k q j", p=128, q=4)
               for w in ("ws", "wd", "whe", "u1", "u2", "w0a", "ow0", "v",
                         "ow1")}
    emb0_v = rows("emb0", 20).rearrange("(a q) j -> a q j", a=5)
    v3b3_v = rows("v3b3", 8).rearrange("(a q) j -> a q j", a=2)
    b2_v = rows("b2", 4).rearrange("(a q) j -> a q j", a=1)
    b4_v = rows("b4", 4).rearrange("(a q) j -> a q j", a=1)
    ob1_v = rows("ob1", 4).rearrange("(a q) j -> a q j", a=1)
    iota_v = rows("iota128", 128)
    ident_v = rows("ident", 128)
    iotacol_v = rows("iotacol", 128)
    cnt_v = rows("cntones", 128).rearrange("(a c q) j -> a c q j", a=2, c=16)
    attrs_v = rows("attrs5", 5 * ECP // 128).rearrange(
        "(a c) j -> a c j", a=5)
    dstrep_v = rows("dstrep", ECP).rearrange("(p c) j -> p c j", p=128)
    dstcol_v = rows("dstcol", NCP).rearrange("(p c) j -> p c j", p=128)
    srci_v = idx16[:IW, :].rearrange("(p c) j -> p c j", p=128)
    iong_v = idx16[IW:IW + 512, :].rearrange("(p c) j -> p c j", p=128)

    with tile.TileContext(nc) as tc:
        with tc.tile_pool(name="const", bufs=1) as cp, \
             tc.tile_pool(name="dram", bufs=1, space="DRAM") as dp, \
             tc.tile_pool(name="io", bufs=3) as io, \
             tc.tile_pool(name="io2", bufs=2) as io2, \
             tc.tile_pool(name="wk", bufs=4) as wk, \
             tc.tile_pool(name="psA", bufs=4, space="PSUM") as psA, \
             tc.tile_pool(name="psAgg", bufs=2, space="PSUM") as psAgg:

            nc.gpsimd.load_library(library_config.mlp)
            regs = {}

            def reg_of(n):
                if n not in regs:
                    regs[n] = nc.gpsimd.to_reg(n)
                return regs[n]

            # ---- DRAM scratch
            meshproj = dp.tile([NM, D], BF16)
            aggH = dp.tile([NGS, D], BF16)
            aggI = dp.tile([NGS, D], BF16)

            # ---- resident constants in SBUF
            def cload(view, shape, dt, tag):
                t = cp.tile(shape, dt, tag=tag)
                nc.sync.dma_start(t[:], view)
                return t

            w_sb = {"ws": cload(w_views["ws"], [128, 4, 4, 128], BF16,
                                 "ws")}
            ones1_sb = cp.tile([1, 128], BF16, tag="ones1")
            nc.vector.memset(ones1_sb[:], 1.0)
            onesrow_sb = cp.tile([1, 512], BF16, tag="onesrow")
            nc.vector.memset(onesrow_sb[:], 1.0)
            gridproj_sb = cp.tile([128, NBLK, D], BF16, tag="gproj")

            # ---- P1+P2 interleaved: mesh_proj = mesh@Ws -> DRAM and
            # grid_proj = grid@Wd + b2 -> SBUF.  Both are DMA-heavy and
            # PE-light; interleaving keeps the PE fed from two streams.
            w_sb["wd"] = cload(w_views["wd"], [128, 4, 4, 128], BF16, "wd")
            b2_sb = cload(b2_v, [1, 4, 128], BF16, "b2")
            NMB = NM // 128
            mT4 = None

            def p1_block(mb):
                nonlocal mT4
                q = mb % 4
                if q == 0:
                    nq = min(4, NMB - mb)
                    mT4 = io2.tile([128, 4, 4, 128], BF16, tag="p1g")
                    nc.sync.dma_start(mT4[:, :, :nq, :],
                                      meshT_v[:, :, mb:mb + nq, :])
                ps = psA.tile([128, D], F32, tag="mm")
                for k in range(4):
                    nc.tensor.matmul(ps[:], mT4[:, k, q, :], w_sb["ws"][:, k],
                                     start=(k == 0), stop=(k == 3))
                mp = io2.tile([128, D], BF16, tag="p1o")
                nc.scalar.copy(mp[:], ps[:])
                nc.scalar.dma_start(meshproj[mb * 128:(mb + 1) * 128, :],
                                    mp[:])

            def p2_block(gb):
                gT = io.tile([128, 4, 128], BF16, tag="p2g")
                nc.sync.dma_start(gT[:], gridT_v[:, :, gb // 4, gb % 4, :])
                ps = psA.tile([128, D], F32, tag="mm")
                for k in range(4):
                    nc.tensor.matmul(ps[:], gT[:, k, :], w_sb["wd"][:, k],
                                     start=(k == 0), stop=False)
                nc.tensor.matmul(ps[:], ones1_sb[:], b2_sb[:, :, :],
                                 start=False, stop=True)
                nc.vector.tensor_copy(gridproj_sb[:, gb, :], ps[:])

            for j in range(NMB):
                p1_block(j)
                if j < NBLK:
                    p2_block(j)

            w_sb["whe"] = cload(w_views["whe"], [128, 4, 4, 128], BF16,
                                "whe")
            emb0_sb = cload(emb0_v, [5, 4, 128], BF16, "emb0")
            iota_sb = cload(iota_v, [128, 128], BF16, "iota")
            iotacol_b = cload(iotacol_v, [128, 128], BF16, "iotacol")
            iotacol_sb = cp.tile([128, 1], F32, tag="iotacolf")
            nc.vector.tensor_copy(iotacol_sb[:], iotacol_b[:, 0:1])
            dstcol_b = cload(dstcol_v, [128, NCP // 128, 128], BF16, "dstcol")
            dstcol_sb = cp.tile([128, NCP // 128, 128], F32, tag="dstcolf")
            nc.vector.tensor_copy(dstcol_sb[:], dstcol_b[:])
            srci_sb = cp.tile([128, IW], I16, tag="srci")
            for g in range(IW // 128):
                nc.sync.dma_start(srci_sb[:, g * 128:(g + 1) * 128],
                                  srci_v[:, g, :])
            v3b3_sb = cload(v3b3_v, [2, 4, 128], BF16, "v3b3")
            b4_sb = cload(b4_v, [1, 4, 128], BF16, "b4")
            ob1_sb = cload(ob1_v, [1, 4, 128], BF16, "ob1")
            iong_sb = cload(iong_v, [128, 4, 128], I16, "iong")
            for w in ("u1", "u2", "w0a", "ow0", "v", "ow1"):
                w_sb[w] = cload(w_views[w], [128, 4, 4, 128], BF16, w)

            # ---- P3: edge phase.  Three stages, software-pipelined so
            # the PE never waits on an activation it just produced:
            #   A(i): h for chunk i (psz/hR, psF/hFa, S_cT)
            #   B(i): pre2/hid2/aggH for chunk i (uses A(i) outputs)
            #   C(i): aggI scatter for chunk i (uses B(i)'s hid2)
            CMAX = max(CAPS)
            sched = []            # (block, chunk-in-block, chunk-global)
            coff = 0
            for b in range(NBLK):
                for c in range(CAPS[b]):
                    sched.append((b, c, coff + c))
                coff += CAPS[b]
            NSCH = len(sched)
            blk_tiles = {}
            ch = {}

            def stageA(i):
                b, c, cc = sched[i]
                if c == 0:
                    CAP = CAPS[b]
                    L = CAP * 128
                    e0 = cc * 128
                    attrs_sb = io.tile([5, CMAX, 128], BF16, tag="attrs")
                    nc.sync.dma_start(attrs_sb[:, :CAP, :],
                                      attrs_v[:, cc:cc + CAP, :])
                    dstrep_sb = io.tile([128, CMAX, 128], BF16, tag="dstrep")
                    nc.sync.dma_start(dstrep_sb[:, :CAP, :],
                                      dstrep_v[:, cc:cc + CAP, :])
                    srcGt = io2.tile([128, CMAX, D], BF16, tag="srcG")
                    nc.gpsimd.dma_gather(
                        srcGt[:, :CAP, :], meshproj[:],
                        srci_sb[:, e0 // 16:(e0 + L) // 16],
                        num_idxs=L, num_idxs_reg=reg_of(L), elem_size=D)
                    aggH_t = psAgg.tile([128, D], F32, tag="aggH")
                    aggI_t = psAgg.tile([128, D], F32, tag="aggI")
                    blk_tiles[b] = (aggH_t, aggI_t, attrs_sb, dstrep_sb,
                                    srcGt)
                _, _, attrs_sb, dstrep_sb, _ = blk_tiles[b]
                a_c = attrs_sb[:, c, :]
                psz = psA.tile([128, D], F32, tag="mm")
                nc.tensor.matmul(psz[:], a_c, emb0_sb[:],
                                 start=True, stop=True)
                hR = wk.tile([128, D], BF16, tag="hR")
                nc.scalar.activation(hR[:], psz[:], AF.Silu)
                psF = psA.tile([128, D], F32, tag="mm")
                for f in range(4):
                    nc.tensor.matmul(psF[:, f * 128:(f + 1) * 128],
                                     emb0_sb[:, f, :], a_c,
                                     start=True, stop=True)
                hFa = wk.tile([128, D], BF16, tag="hFa")
                nc.scalar.activation(hFa[:], psF[:], AF.Silu)
                S_cT = wk.tile([128, 128], BF16, tag="S_cT")
                nc.vector.tensor_scalar(
                    S_cT[:], dstrep_sb[:, c, :], iotacol_sb[:, 0:1],
                    None, op0=ALU.is_equal)
                S_c = wk.tile([128, 128], BF16, tag="S_c")
                nc.vector.tensor_scalar(
                    S_c[:], iota_sb[:],
                    dstcol_sb[:, cc // 128, cc % 128:cc % 128 + 1],
                    None, op0=ALU.is_equal)
                ch[i] = dict(hR=hR, hFa=hFa, S_cT=S_cT, S_c=S_c)

            def stageB(i):
                b, c, cc = sched[i]
                aggH_ps, _, _, _, srcGt = blk_tiles[b]
                s = ch[i]
                ps2 = psA.tile([128, D], F32, tag="mm")
                for f in range(4):
                    nc.tensor.matmul(ps2[:],
                                     s["hFa"][:, f * 128:(f + 1) * 128],
                                     w_sb["whe"][:, f],
                                     start=(f == 0), stop=False)
                nc.tensor.matmul(ps2[:], s["S_cT"][:], gridproj_sb[:, b, :],
                                 start=False, stop=True)
                t2 = wk.tile([128, D], BF16, tag="t2")
                nc.vector.tensor_tensor(t2[:], srcGt[:, c, :], ps2[:],
                                        op=ALU.add)
                hid2 = wk.tile([128, D], BF16, tag="hid2")
                nc.scalar.activation(hid2[:], t2[:], AF.Silu)
                nc.tensor.matmul(aggH_ps[:], s["S_c"][:], s["hR"][:],
                                 start=(c == 0), stop=(c == CAPS[b] - 1),
                                 skip_group_check=True)
                if c == CAPS[b] - 1:
                    aH = io2.tile([128, D], BF16, tag="aH")
                    nc.vector.tensor_copy(aH[:], aggH_ps[:])
                    nc.scalar.dma_start(aggH[b * 128:(b + 1) * 128, :], aH[:])
                s["hid2"] = hid2

            def stageC(i):
                b, c, cc = sched[i]
                aggI_ps = blk_tiles[b][1]
                s = ch.pop(i)
                nc.tensor.matmul(aggI_ps[:], s["S_c"][:], s["hid2"][:],
                                 start=(c == 0), stop=(c == CAPS[b] - 1),
                                 skip_group_check=True)
                if c == CAPS[b] - 1:
                    aI = io2.tile([128, D], BF16, tag="aI")
                    nc.vector.tensor_copy(aI[:], aggI_ps[:])
                    nc.scalar.dma_start(aggI[b * 128:(b + 1) * 128, :], aI[:])

            for i in range(NSCH + 3):
                if i < NSCH:
                    stageA(i)
                if 2 <= i <= NSCH + 1:
                    stageB(i - 2)
                if i >= 3:
                    stageC(i - 3)

            # ---- P4: node + output MLPs, 512-row blocks, staged so the
            # PE never waits on the silu of the stage it just fed
            NRB = NGS // 512
            st = {}

            def p4_load(rb):
                cnt_sb = io.tile([2, 4, 128], BF16, tag="cnt")
                nc.sync.dma_start(cnt_sb[:], cnt_v[:, rb, :, :])
                gT = io2.tile([128, 4, 4, 128], BF16, tag="gT4")
                nc.sync.dma_start(gT[:], gridT_v[:, :, rb, :, :])
                isl = iong_sb[:, rb // 4, (rb % 4) * 32:(rb % 4) * 32 + 32]
                aHT = io2.tile([128, 4, D], BF16, tag="aHT")
                nc.gpsimd.dma_gather(aHT[:], aggH[:], isl,
                                     num_idxs=512, num_idxs_reg=reg_of(512),
                                     elem_size=D, transpose=True)
                aIT = io2.tile([128, 4, D], BF16, tag="aIT")
                nc.gpsimd.dma_gather(aIT[:], aggI[:], isl,
                                     num_idxs=512, num_idxs_reg=reg_of(512),
                                     elem_size=D, transpose=True)
                st[rb] = dict(cnt=cnt_sb, gT=gT, aHT=aHT, aIT=aIT)

            def p4_h3(rb):
                s = st[rb]
                h3 = io2.tile([128, 4, 512], BF16, tag="h3")
                for g in range(4):
                    ps3 = psA.tile([128, 512], F32, tag="mm")
                    for k in range(4):
                        nc.tensor.matmul(ps3[:], w_sb["w0a"][:, k, g, :],
                                         s["gT"][:, k], start=(k == 0),
                                         stop=False)
                    for k in range(4):
                        nc.tensor.matmul(ps3[:], w_sb["u1"][:, k, g, :],
                                         s["aHT"][:, k, :], start=False,
                                         stop=False)
                    for k in range(4):
                        nc.tensor.matmul(ps3[:], w_sb["u2"][:, k, g, :],
                                         s["aIT"][:, k, :], start=False,
                                         stop=False)
                    nc.tensor.matmul(ps3[:], v3b3_sb[:, g, :], s["cnt"][:],
                                     start=False, stop=True)
                    nc.scalar.activation(h3[:, g, :], ps3[:], AF.Silu)
                s["h3"] = h3

            def p4_h4(rb):
                s = st[rb]
                h4 = io2.tile([128, 4, 512], BF16, tag="h4")
                for g in range(4):
                    ps4 = psA.tile([128, 512], F32, tag="mm")
                    for k in range(4):
                        nc.tensor.matmul(ps4[:], w_sb["ow0"][:, k, g, :],
                                         s["gT"][:, k], start=(k == 0),
                                         stop=False)
                    for k in range(4):
                        nc.tensor.matmul(ps4[:], w_sb["v"][:, k, g, :],
                                         s["h3"][:, k, :], start=False,
                                         stop=False)
                    nc.tensor.matmul(ps4[:], b4_sb[:, g, :], onesrow_sb[:],
                                     start=False, stop=True)
                    nc.scalar.activation(h4[:, g, :], ps4[:], AF.Silu)
                s["h4"] = h4

            def p4_out(rb):
                s = st.pop(rb)
                r0 = rb * 512
                for sc in range(4):
                    rs = slice(sc * 128, (sc + 1) * 128)
                    pso = psA.tile([128, 512], F32, tag="mm")
                    for k in range(4):
                        nc.tensor.matmul(pso[:], s["h4"][:, k, rs],
                                         w_sb["ow1"][:, k],
                                         start=(k == 0), stop=False)
                    nc.tensor.matmul(pso[:], ones1_sb[:], ob1_sb[:],
                                     start=False, stop=True)
                    ot = io2.tile([128, 512], BF16, tag="ot")
                    nc.vector.tensor_copy(ot[:], pso[:])
                    nc.scalar.dma_start(
                        outt[r0 + sc * 128:r0 + (sc + 1) * 128, :],
                        ot[:, :OUTD])

            for rb in range(NRB + 2):
                if rb < NRB:
                    p4_load(rb)
                    p4_h3(rb)
                if 1 <= rb <= NRB:
                    p4_h4(rb - 1)
                if rb >= 2:
                    p4_out(rb - 2)

    from concourse.library_overlay import lower_extended_insts
    lower_extended_insts(nc)   # fill .instr of InstISA subclasses (load_library)
    if SPLIT_WAITS:
        _split_multi_waits(nc)
    return nc


def _split_multi_waits(nc):
    """This walrus build allows at most ONE sync wait per instruction.
    Move surplus waits onto EventSemaphore carrier instructions inserted
    immediately before, on the same engine (semantically identical: the
    sequencer blocks on each in order)."""
    for f in nc.m.functions:
        for bb in f.blocks:
            insts = list(bb.instructions)
            if not any(i.sync_info is not None and len(i.sync_info.on_wait) > 1
                       for i in insts):
                continue
            new = []
            for ins in insts:
                si = ins.sync_info
                if si is not None and len(si.on_wait) > 1:
                    waits = list(si.on_wait)
                    for w in waits[:-1]:
                        c = mybir.InstEventSemaphore(
                            name=f"I-w{nc.next_id()}", engine=ins.engine,
                            ins=[], outs=[],
                            sync_info=mybir.SyncInfo(on_wait=[w], on_update=[]))
                        new.append(c)
                    del si.on_wait[:]
                    si.on_wait.append(waits[-1])
                new.append(ins)
            bb.instructions = new


# ------------------------------------------------------------ host pipeline
def _prep(inputs):
    """Host-side fold/sort/pack. Returns (in_maps, CAPS, perm)."""
    mesh_f = np.asarray(inputs["mesh_node_features"])[0]   # [N_MESH, D]
    grid_f = np.asarray(inputs["grid_node_features"])[0]   # [N_GRID, D]
    attrs = np.asarray(inputs["edge_attrs"])               # [E, 4]
    esrc = np.asarray(inputs["edge_src"]).astype(np.int64)
    edst = np.asarray(inputs["edge_dst"]).astype(np.int64)

    # ---- fold weights (fp32 on host, cast bf16)
    W = {k: np.asarray(inputs[k], np.float32) for k in (
        "emb_w0", "emb_b0", "emb_w1", "emb_b1", "edge_w0", "edge_b0",
        "edge_w1", "edge_b1", "node_w0", "node_b0", "node_w1", "node_b1",
        "out_w0", "out_b0", "out_w1", "out_b1")}
    Ws, Wd, We = W["edge_w0"][:D], W["edge_w0"][D:2 * D], W["edge_w0"][2 * D:]
    W0a, W0b = W["node_w0"][:D], W["node_w0"][D:]
    W_he = W["emb_w1"] @ We
    b2 = W["emb_b1"] @ We + W["edge_b0"]
    U1 = W["emb_w1"] @ W0b
    U2 = W["edge_w1"] @ W0b
    v3 = (W["emb_b1"] + W["edge_b1"]) @ W0b
    V = W["node_w1"] @ W["out_w0"]
    b4 = W["node_b1"] @ W["out_w0"] + W["out_b0"]
    emb_w0b = np.concatenate([W["emb_w0"], W["emb_b0"][None]], 0)  # [5, D]
    v3b3 = np.stack([v3, W["node_b0"]], 0)                          # [2, D]
    ow1p = np.zeros((D, 512), np.float32)
    ow1p[:, :OUTD] = W["out_w1"]
    ob1p = np.zeros(512, np.float32)
    ob1p[:OUTD] = W["out_b1"]

    # ---- sort edges by global dst block, balance blocks across cores
    order = np.argsort(edst, kind="stable")
    esrc, edst, attrs = esrc[order], edst[order], attrs[order]
    gblk = edst // 128                       # global 128-row dst block
    counts = np.bincount(gblk, minlength=NGB)
    blk_rank = np.argsort(counts)[::-1]      # descending edge count
    # slot assignment: rank r -> core r%8, slot r//8 (pad to 512 with -1)
    blocks = np.full(NCORES * NBLK, -1, np.int64)
    blocks[:NGB] = blk_rank
    core_of_rank = np.arange(NCORES * NBLK) % NCORES
    slot_of_rank = np.arange(NCORES * NBLK) // NCORES
    CAPS = np.zeros(NBLK, np.int64)
    for s in range(NBLK):
        gbs = blocks[s * NCORES:(s + 1) * NCORES]
        mx = max(int(counts[g]) if g >= 0 else 0 for g in gbs)
        CAPS[s] = max(1, _cdiv(mx, 128))
    CAPS = tuple(int(x) for x in CAPS)
    NCHUNK = sum(CAPS)
    ECP = NCHUNK * 128
    NCP = _pad128(NCHUNK)
    IW = _pad128(ECP // 16)
    chunk_off = np.concatenate([[0], np.cumsum(CAPS)])

    # per-core block table: core c slot s -> global block id (or -1)
    blk_of = np.full((NCORES, NBLK), -1, np.int64)
    for rnk in range(NCORES * NBLK):
        blk_of[core_of_rank[rnk], slot_of_rank[rnk]] = blocks[rnk]

    meshT = np.zeros((D, NM), np.float32)
    meshT[:, :N_MESH] = mesh_f.T
    iotaNG = _wrap_idx(np.arange(NGS))
    iota128 = np.tile(np.arange(128, dtype=np.float32)[None], (128, 1))
    iotacol = np.zeros((128, 128), np.float32)
    iotacol[:, 0] = np.arange(128)

    shared_parts = []

    def add(name, arr):
        shared_parts.append((name, np.asarray(arr, np.float32)))

    add("meshT", meshT.reshape(-1, 128))
    # gridT placeholder per core (inserted at same offset per core below)
    add("gridT", np.zeros((NGS * 4, 128), np.float32))
    for nm_, w_ in (("ws", Ws), ("wd", Wd), ("whe", W_he), ("u1", U1),
                    ("u2", U2), ("w0a", W0a), ("ow0", W["out_w0"]),
                    ("v", V), ("ow1", ow1p)):
        add(nm_, w_.reshape(-1, 128))
    add("emb0", emb_w0b.reshape(-1, 128))
    add("v3b3", v3b3.reshape(-1, 128))
    add("b2", b2.reshape(-1, 128))
    add("b4", b4.reshape(-1, 128))
    add("ob1", ob1p.reshape(-1, 128))
    add("iota128", iota128)
    add("ident", np.eye(128, dtype=np.float32))
    add("iotacol", iotacol)
    add("cntones", np.zeros((128, 128), np.float32))   # per-core below
    add("attrs5", np.zeros((5 * ECP // 128, 128), np.float32))
    add("dstrep", np.zeros((ECP, 128), np.float32))
    add("dstcol", np.zeros((NCP, 128), np.float32))

    names = [n for n, _ in shared_parts]
    base_arrays = {n: a for n, a in shared_parts}
    per_core_names = {"gridT", "cntones", "attrs5", "dstrep", "dstcol"}

    in_maps = []
    perm = []            # (core, local_row0, global_row0, nrows)
    for core in range(NCORES):
        src_p = np.zeros(ECP, np.int64)
        dib_p = np.full(ECP, 999.0, np.float32)
        att_p = np.zeros((ECP, 4), np.float32)
        grid_b = np.zeros((NGS, D), np.float32)
        cnt = np.zeros(NGS, np.float32)
        for s in range(NBLK):
            gb = blk_of[core, s]
            if gb < 0:
                continue
            g0 = gb * 128
            g1 = min(g0 + 128, N_GRID)
            sz = g1 - g0
            grid_b[s * 128:s * 128 + sz] = grid_f[g0:g1]
            perm.append((core, s * 128, g0, sz))
            m = gblk == gb
            n = int(m.sum())
            o = chunk_off[s] * 128
            assert n <= CAPS[s] * 128
            src_p[o:o + n] = esrc[m]
            dloc = (edst[m] - g0).astype(np.float32)
            dib_p[o:o + n] = dloc
            att_p[o:o + n] = attrs[m]
            cnt[s * 128:s * 128 + sz] = np.bincount(
                dloc.astype(np.int64), minlength=128)[:sz]

        attrsT5 = np.concatenate(
            [att_p.T, np.ones((1, ECP), np.float32)], 0)       # [5, ECP]
        dstrep = np.tile(dib_p[None], (128, 1))                 # [128, ECP]
        dstcol = np.zeros((128, NCP), np.float32)
        dstcol[:, :NCHUNK] = np.ascontiguousarray(
            dib_p.reshape(-1, 128).T)
        cntones = np.zeros((2, NGS), np.float32)
        cntones[0] = cnt
        cntones[1] = 1.0

        parts = []
        for n in names:
            if n == "gridT":
                parts.append(grid_b.T.reshape(-1, 128))
            elif n == "cntones":
                parts.append(cntones.reshape(-1, 128))
            elif n == "attrs5":
                parts.append(attrsT5.reshape(-1, 128))
            elif n == "dstrep":
                parts.append(dstrep.reshape(-1, 128))
            elif n == "dstcol":
                parts.append(dstcol.reshape(-1, 128))
            else:
                parts.append(base_arrays[n])
        blob = np.concatenate([p.astype(bf) for p in parts], 0)

        srcw = np.zeros((128, IW), np.int16)
        srcw[:, :ECP // 16] = _wrap_idx(src_p)
        idx = np.concatenate([np.ascontiguousarray(srcw).reshape(-1, 128),
                              iotaNG.reshape(-1, 128)], 0).astype(np.int16)
        in_maps.append({"blob": np.ascontiguousarray(blob),
                        "idx16": np.ascontiguousarray(idx)})
    return in_maps, CAPS, perm


_CACHE = {}


class _Runner:
    """Persistent jitted SPMD executor (avoids re-jitting per call)."""

    def __init__(self, nc):
        import jax
        from jax.experimental.shard_map import shard_map
        from jax.sharding import Mesh, PartitionSpec
        from concourse import bass2jax

        bass2jax.install_neuronx_cc_hook()
        self.nc = nc
        part_name = (nc.partition_id_tensor.name
                     if nc.partition_id_tensor else None)
        in_names, out_names, out_avals, zero_outs = [], [], [], []
        for alloc in nc.m.functions[0].allocations:
            if not isinstance(alloc, mybir.MemoryLocationSet):
                continue
            name = alloc.memorylocations[0].name
            if alloc.kind == "ExternalInput":
                if name != part_name:
                    in_names.append(name)
            elif alloc.kind == "ExternalOutput":
                shape = tuple(alloc.tensor_shape)
                dtype = mybir.dt.np(alloc.dtype)
                out_names.append(name)
                out_avals.append(jax.core.ShapedArray(shape, dtype))
                zero_outs.append(np.zeros(shape, dtype))
        self.in_names = list(in_names)
        self.out_names = out_names
        self.out_shapes = [tuple(a.shape) for a in out_avals]
        all_names = in_names + out_names
        if part_name is not None:
            all_names = all_names + [part_name]

        def _body(*args):
            operands = list(args)
            if part_name is not None:
                operands.append(bass2jax.partition_id_tensor())
            outs = bass2jax._bass_exec_p.bind(
                *operands,
                out_avals=tuple(out_avals),
                in_names=tuple(all_names),
                out_names=tuple(out_names),
                lowering_input_output_aliases=(),
                sim_require_finite=True,
                sim_require_nnan=True,
                nc=nc,
            )
            return tuple(outs)

        devices = jax.devices()[:NCORES]
        mesh = Mesh(np.asarray(devices), ("core",))
        nin = len(self.in_names) + len(out_names)
        self.fn = jax.jit(shard_map(
            _body, mesh=mesh,
            in_specs=(PartitionSpec("core"),) * nin,
            out_specs=(PartitionSpec("core"),) * len(out_names),
            check_rep=False))
        self.zero_outs = zero_outs
        self.sharding = jax.sharding.NamedSharding(mesh, PartitionSpec("core"))
        self.mesh = mesh
        self._avals = out_avals
        self._jax = jax

    def put(self, in_maps):
        """Concatenate per-core inputs on axis 0, device_put pre-sharded."""
        arrs = []
        for name in self.in_names:
            arrs.append(np.concatenate([m[name] for m in in_maps], axis=0))
        for z in self.zero_outs:
            arrs.append(np.concatenate([z] * NCORES, axis=0))
        return [self._jax.device_put(a, self.sharding) for a in arrs]

    def run(self, arrs):
        return self.fn(*arrs)

    def get(self, outs):
        res = [np.asarray(o) for o in outs]
        per_core = []
        for c in range(NCORES):
            d = {}
            for i, name in enumerate(self.out_names):
                n0 = self.out_shapes[i][0]
                d[name] = res[i][c * n0:(c + 1) * n0]
            per_core.append(d)
        return per_core


def _get_runner(CAPS) -> _Runner:
    if CAPS not in _CACHE:
        _CACHE[CAPS] = _Runner(build_bass(CAPS))
    return _CACHE[CAPS]


def kernel(**inputs) -> np.ndarray:
    in_maps, CAPS, perm = _prep(inputs)
    r = _get_runner(CAPS)
    outs = r.run(r.put(in_maps))
    per_core = r.get(outs)
    out = np.zeros((N_GRID, OUTD), np.float32)
    for core, l0, g0, sz in perm:
        o = per_core[core]["outt"]
        out[g0:g0 + sz] = o[l0:l0 + sz].astype(np.float32)
    return out[None]


# revision 31
# speedup vs baseline: 1.9829x; 1.1476x over previous
"""Trainium2 Bass kernel for Mesh2GridDecoder (GraphCast-style mesh->grid
message passing + output MLP), distributed over 8 NeuronCores.

Strategy: grid nodes are sharded across cores by 128-row destination blocks
(blocks load-balanced across cores by edge count so the scatter-sum is
core-local); mesh node features and all weights are replicated.  Everything
runs in bf16 with fp32 PSUM accumulation.

Math restructuring (exact, up to float re-association):
  h     = silu(attrs @ emb_w0 + emb_b0)                       per edge
  pre2  = src@Ws + dst@Wd + e_emb@We + edge_b0
        = mesh_proj[src] + grid_proj[dst] + h @ W_he
    with mesh_proj = mesh@Ws, grid_proj = grid@Wd + (emb_b1@We + edge_b0),
         W_he = emb_w1 @ We
  hid2  = silu(pre2)
  agg   = (S@h)@emb_w1 + (S@hid2)@edge_w1 + cnt (x) (emb_b1+edge_b1)
  pre3  = grid@W0a + (S@h)@U1 + (S@hid2)@U2 + cnt (x) v3 + node_b0
  hid3  = silu(pre3)
  pre4  = grid@out_w0 + hid3@V + b4
  out   = silu(pre4) @ out_w1 + out_b1

Edges are sorted by dst and grouped into 128-row dst blocks.  Per 128-edge
chunk a 0/1 selector S_c[e,d] = (dst_in_block[e]==d) drives the scatter-sum
on the tensor engine; its transpose S_cT[d,e] turns the dst-feature gather
grid_proj[dst] into a matmul accumulated straight into the pre2 PSUM.
mesh/grid arrive pre-transposed from the host so no on-device transposes are
needed outside the aggregation read-back.
"""
import math
import numpy as np
import ml_dtypes

import concourse.bass as bass
import concourse.tile as tile
from concourse import mybir
from concourse import bass_utils
from concourse import library_config
from concourse.vector_clock import ScopedClock

BF16 = mybir.dt.bfloat16
F32 = mybir.dt.float32
I16 = mybir.dt.int16
AF = mybir.ActivationFunctionType
ALU = mybir.AluOpType
bf = ml_dtypes.bfloat16

N_MESH = 10242
N_GRID = 65160
N_EDGES = 195480
D = 512
OUTD = 471
NCORES = 8
NM = 10368                      # padded mesh rows (81 chunks of 128)
NGB = (N_GRID + 127) // 128     # 510 global 128-row dst blocks
NBLK = 64                       # dst blocks per core (8*64 = 512 slots)
NGS = NBLK * 128                # 8192 local grid rows per core
SPLIT_WAITS = True              # walrus 1-wait/inst workaround (off for CoreSim)


# ---------------------------------------------------------------- tile patch
def _patched_drain_and_barrier(self, tick_clock, wait_clock):
    # This walrus build accepts at most 1 sync wait per instruction; the
    # stock tail drain carries one wait per active proc.  Emit explicit
    # wait_ge instructions instead.
    probe = self.nc.sync.nop()
    if probe.ins.sync_info is None:
        probe.ins.sync_info = mybir.SyncInfo(on_wait=[], on_update=[])
    wait_clock.add_sem_waits(probe.ins, ScopedClock({None: tick_clock.global_clock}))
    waits = list(probe.ins.sync_info.on_wait)
    del probe.ins.sync_info.on_wait[:]
    name2sem = {s.name: s for s in self.sems.allocated().values()}
    for w in waits:
        self.nc.sync.wait_ge(name2sem[w.ant_name], w.wait_value)
    self.nc.sync.drain()
    self.nc.all_engine_barrier()
    assert self.sems is not None
    popped = self.nc._tile_sem_poison_stack.pop()
    assert popped is self._sem_poison
    self.nc.clear_and_free_semaphores(list(self.sems.allocated().values()))
    self.nc.all_engine_barrier()


tile.TileContext._drain_and_barrier = _patched_drain_and_barrier


# ------------------------------------------------------------------- helpers
def _wrap_idx(idx: np.ndarray) -> np.ndarray:
    """dma_gather index layout: index i at [i % 16, i // 16], the 16-row
    block replicated down all 128 partitions."""
    assert idx.size % 16 == 0
    w = idx.astype(np.int16).reshape(-1, 16).T  # [16, n/16]
    return np.ascontiguousarray(np.tile(w, (8, 1)))


def _cdiv(a, b):
    return (a + b - 1) // b


def _pad128(n):
    return _cdiv(n, 128) * 128


def build_bass(CAPS):
    """Build the per-core Bass program.  CAPS[b] = 128-edge chunks in dst
    block b (same for every core)."""
    CAPS = list(CAPS)
    NCHUNK = sum(CAPS)
    ECP = NCHUNK * 128
    NCP = _pad128(NCHUNK)          # padded chunk-count (dstb_col width)
    IW = _pad128(ECP // 16)        # padded srcidx width

    # blob row offsets (must match _prep's packing order)
    offs = {}
    r = 0

    def takerows(name, nrows):
        nonlocal r
        offs[name] = r
        r += nrows

    takerows("meshT", NM * 4)            # [512, NM] -> (f*81+b, j) rows
    takerows("gridT", NGS * 4)           # [512, NGS] -> ((k*128+p)*64 + c) rows
    for w in ("ws", "wd", "whe", "u1", "u2", "w0a", "ow0", "v", "ow1"):
        takerows(w, 2048)                # [512, 512] -> ((k*128+p)*4+q, j)
    takerows("emb0", 20)                 # [5, 512]
    takerows("v3b3", 8)                  # [2, 512]
    takerows("b2", 4)                    # [1, 512]
    takerows("b4", 4)
    takerows("ob1", 4)
    takerows("iota128", 128)             # [128, 128]
    takerows("ident", 128)               # [128, 128] eye
    takerows("iotacol", 128)             # [128, 128] (col 0 = arange)
    takerows("cntones", 128)             # [2, 8192]
    takerows("attrs5", 5 * ECP // 128)   # [5, ECP]
    takerows("dstrep", ECP)              # [128, ECP]
    takerows("dstcol", NCP)              # [128, NCP]
    RBLOB = r

    nc = bass.Bass("TRN2", target_bir_lowering=False, debug=False,
                   num_devices=NCORES)

    blob = nc.dram_tensor("blob", [RBLOB, 128], BF16, kind="ExternalInput").ap()
    idx16 = nc.dram_tensor("idx16", [IW + 512, 128], I16,
                           kind="ExternalInput").ap()
    outt = nc.dram_tensor("outt", [NGS, OUTD], BF16,
                          kind="ExternalOutput").ap()

    def rows(name, n):
        o = offs[name]
        return blob[o:o + n, :]

    meshT_v = rows("meshT", NM * 4).rearrange("(k p b) j -> p k b j",
                                              p=128, b=81)
    gridT_v = rows("gridT", NGS * 4).rearrange("(k p c q) j -> p k c q j",
                                               p=128, c=16, q=4)
    w_views = {w: rows(w, 2048).rearrange("(k p q) j -> p k q j", p=128, q=4)
               for w in ("ws", "wd", "whe", "u1", "u2", "w0a", "ow0", "v",
                         "ow1")}
    emb0_v = rows("emb0", 20).rearrange("(a q) j -> a q j", a=5)
    v3b3_v = rows("v3b3", 8).rearrange("(a q) j -> a q j", a=2)
    b2_v = rows("b2", 4).rearrange("(a q) j -> a q j", a=1)
    b4_v = rows("b4", 4).rearrange("(a q) j -> a q j", a=1)
    ob1_v = rows("ob1", 4).rearrange("(a q) j -> a q j", a=1)
    iota_v = rows("iota128", 128)
    ident_v = rows("ident", 128)
    iotacol_v = rows("iotacol", 128)
    cnt_v = rows("cntones", 128).rearrange("(a c q) j -> a c q j", a=2, c=16)
    attrs_v = rows("attrs5", 5 * ECP // 128).rearrange(
        "(a c) j -> a c j", a=5)
    dstrep_v = rows("dstrep", ECP).rearrange("(p c) j -> p c j", p=128)
    dstcol_v = rows("dstcol", NCP).rearrange("(p c) j -> p c j", p=128)
    srci_v = idx16[:IW, :].rearrange("(p c) j -> p c j", p=128)
    iong_v = idx16[IW:IW + 512, :].rearrange("(p c) j -> p c j", p=128)

    with tile.TileContext(nc) as tc:
        with tc.tile_pool(name="const", bufs=1) as cp, \
             tc.tile_pool(name="dram", bufs=1, space="DRAM") as dp, \
             tc.tile_pool(name="io", bufs=3) as io, \
             tc.tile_pool(name="io2", bufs=2) as io2, \
             tc.tile_pool(name="wk", bufs=4) as wk, \
             tc.tile_pool(name="psA", bufs=4, space="PSUM") as psA, \
             tc.tile_pool(name="psAgg", bufs=2, space="PSUM") as psAgg:

            nc.gpsimd.load_library(library_config.mlp)
            regs = {}

            def reg_of(n):
                if n not in regs:
                    regs[n] = nc.gpsimd.to_reg(n)
                return regs[n]

            # ---- DRAM scratch
            meshproj = dp.tile([NM, D], BF16)
            aggH = dp.tile([NGS, D], BF16)
            aggI = dp.tile([NGS, D], BF16)

            # ---- resident constants in SBUF
            def cload(view, shape, dt, tag):
                t = cp.tile(shape, dt, tag=tag)
                nc.sync.dma_start(t[:], view)
                return t

            w_sb = {"ws": cload(w_views["ws"], [128, 4, 4, 128], BF16,
                                 "ws")}
            ones1_sb = cp.tile([1, 128], BF16, tag="ones1")
            nc.vector.memset(ones1_sb[:], 1.0)
            onesrow_sb = cp.tile([1, 512], BF16, tag="onesrow")
            nc.vector.memset(onesrow_sb[:], 1.0)
            gridproj_sb = cp.tile([128, NBLK, D], BF16, tag="gproj")

            # ---- P1+P2 interleaved: mesh_proj = mesh@Ws -> DRAM and
            # grid_proj = grid@Wd + b2 -> SBUF.  Both are DMA-heavy and
            # PE-light; interleaving keeps the PE fed from two streams.
            w_sb["wd"] = cload(w_views["wd"], [128, 4, 4, 128], BF16, "wd")
            b2_sb = cload(b2_v, [1, 4, 128], BF16, "b2")
            NMB = NM // 128
            mT4 = None

            def p1_block(mb):
                nonlocal mT4
                q = mb % 4
                if q == 0:
                    nq = min(4, NMB - mb)
                    mT4 = io2.tile([128, 4, 4, 128], BF16, tag="p1g")
                    nc.sync.dma_start(mT4[:, :, :nq, :],
                                      meshT_v[:, :, mb:mb + nq, :])
                ps = psA.tile([128, D], F32, tag="mm")
                for k in range(4):
                    nc.tensor.matmul(ps[:], mT4[:, k, q, :], w_sb["ws"][:, k],
                                     start=(k == 0), stop=(k == 3))
                mp = io2.tile([128, D], BF16, tag="p1o")
                nc.scalar.copy(mp[:], ps[:])
                nc.scalar.dma_start(meshproj[mb * 128:(mb + 1) * 128, :],
                                    mp[:])

            def p2_block(gb):
                gT = io.tile([128, 4, 128], BF16, tag="p2g")
                nc.sync.dma_start(gT[:], gridT_v[:, :, gb // 4, gb % 4, :])
                ps = psA.tile([128, D], F32, tag="mm")
                for k in range(4):
                    nc.tensor.matmul(ps[:], gT[:, k, :], w_sb["wd"][:, k],
                                     start=(k == 0), stop=False)
                nc.tensor.matmul(ps[:], ones1_sb[:], b2_sb[:, :, :],
                                 start=False, stop=True)
                nc.vector.tensor_copy(gridproj_sb[:, gb, :], ps[:])

            for j in range(NMB):
                p1_block(j)
                if j < NBLK:
                    p2_block(j)

            w_sb["whe"] = cload(w_views["whe"], [128, 4, 4, 128], BF16,
                                "whe")
            emb0_sb = cload(emb0_v, [5, 4, 128], BF16, "emb0")
            iota_sb = cload(iota_v, [128, 128], BF16, "iota")
            iotacol_b = cload(iotacol_v, [128, 128], BF16, "iotacol")
            iotacol_sb = cp.tile([128, 1], F32, tag="iotacolf")
            nc.vector.tensor_copy(iotacol_sb[:], iotacol_b[:, 0:1])
            dstcol_b = cload(dstcol_v, [128, NCP // 128, 128], BF16, "dstcol")
            dstcol_sb = cp.tile([128, NCP // 128, 128], F32, tag="dstcolf")
            nc.vector.tensor_copy(dstcol_sb[:], dstcol_b[:])
            srci_sb = cp.tile([128, IW], I16, tag="srci")
            for g in range(IW // 128):
                nc.sync.dma_start(srci_sb[:, g * 128:(g + 1) * 128],
                                  srci_v[:, g, :])
            v3b3_sb = cload(v3b3_v, [2, 4, 128], BF16, "v3b3")
            b4_sb = cload(b4_v, [1, 4, 128], BF16, "b4")
            ob1_sb = cload(ob1_v, [1, 4, 128], BF16, "ob1")
            iong_sb = cload(iong_v, [128, 4, 128], I16, "iong")
            for w in ("u1", "u2", "w0a", "ow0", "v", "ow1"):
                w_sb[w] = cload(w_views[w], [128, 4, 4, 128], BF16, w)

            # ---- P3: edge phase.  Three stages, software-pipelined so
            # the PE never waits on an activation it just produced:
            #   A(i): h for chunk i (psz/hR, psF/hFa, S_cT)
            #   B(i): pre2/hid2/aggH for chunk i (uses A(i) outputs)
            #   C(i): aggI scatter for chunk i (uses B(i)'s hid2)
            CMAX = max(CAPS)
            sched = []            # (block, chunk-in-block, chunk-global)
            coff = 0
            for b in range(NBLK):
                for c in range(CAPS[b]):
                    sched.append((b, c, coff + c))
                coff += CAPS[b]
            NSCH = len(sched)
            blk_tiles = {}
            ch = {}

            def stageA(i):
                b, c, cc = sched[i]
                if c == 0:
                    CAP = CAPS[b]
                    L = CAP * 128
                    e0 = cc * 128
                    attrs_sb = io.tile([5, CMAX, 128], BF16, tag="attrs")
                    nc.sync.dma_start(attrs_sb[:, :CAP, :],
                                      attrs_v[:, cc:cc + CAP, :])
                    dstrep_sb = io.tile([128, CMAX, 128], BF16, tag="dstrep")
                    nc.sync.dma_start(dstrep_sb[:, :CAP, :],
                                      dstrep_v[:, cc:cc + CAP, :])
                    srcGt = io2.tile([128, CMAX, D], BF16, tag="srcG")
                    nc.gpsimd.dma_gather(
                        srcGt[:, :CAP, :], meshproj[:],
                        srci_sb[:, e0 // 16:(e0 + L) // 16],
                        num_idxs=L, num_idxs_reg=reg_of(L), elem_size=D)
                    aggH_t = psAgg.tile([128, D], F32, tag="aggH")
                    aggI_t = psAgg.tile([128, D], F32, tag="aggI")
                    blk_tiles[b] = (aggH_t, aggI_t, attrs_sb, dstrep_sb,
                                    srcGt)
                _, _, attrs_sb, dstrep_sb, _ = blk_tiles[b]
                a_c = attrs_sb[:, c, :]
                psz = psA.tile([128, D], F32, tag="mm")
                nc.tensor.matmul(psz[:], a_c, emb0_sb[:],
                                 start=True, stop=True)
                hR = wk.tile([128, D], BF16, tag="hR")
                nc.scalar.activation(hR[:], psz[:], AF.Silu)
                psF = psA.tile([128, D], F32, tag="mm")
                for f in range(4):
                    nc.tensor.matmul(psF[:, f * 128:(f + 1) * 128],
                                     emb0_sb[:, f, :], a_c,
                                     start=True, stop=True)
                hFa = wk.tile([128, D], BF16, tag="hFa")
                nc.scalar.activation(hFa[:], psF[:], AF.Silu)
                S_cT = wk.tile([128, 128], BF16, tag="S_cT")
                nc.vector.tensor_scalar(
                    S_cT[:], dstrep_sb[:, c, :], iotacol_sb[:, 0:1],
                    None, op0=ALU.is_equal)
                S_c = wk.tile([128, 128], BF16, tag="S_c")
                nc.vector.tensor_scalar(
                    S_c[:], iota_sb[:],
                    dstcol_sb[:, cc // 128, cc % 128:cc % 128 + 1],
                    None, op0=ALU.is_equal)
                ch[i] = dict(hR=hR, hFa=hFa, S_cT=S_cT, S_c=S_c)

            def stageB(i):
                b, c, cc = sched[i]
                aggH_ps, _, _, _, srcGt = blk_tiles[b]
                s = ch[i]
                ps2 = psA.tile([128, D], F32, tag="mm")
                for f in range(4):
                    nc.tensor.matmul(ps2[:],
                                     s["hFa"][:, f * 128:(f + 1) * 128],
                                     w_sb["whe"][:, f],
                                     start=(f == 0), stop=False)
                nc.tensor.matmul(ps2[:], s["S_cT"][:], gridproj_sb[:, b, :],
                                 start=False, stop=True)
                t2 = wk.tile([128, D], BF16, tag="t2")
                nc.vector.tensor_tensor(t2[:], srcGt[:, c, :], ps2[:],
                                        op=ALU.add)
                hid2 = wk.tile([128, D], BF16, tag="hid2")
                nc.scalar.activation(hid2[:], t2[:], AF.Silu)
                nc.tensor.matmul(aggH_ps[:], s["S_c"][:], s["hR"][:],
                                 start=(c == 0), stop=(c == CAPS[b] - 1),
                                 skip_group_check=True)
                if c == CAPS[b] - 1:
                    aH = io2.tile([128, D], BF16, tag="aH")
                    nc.vector.tensor_copy(aH[:], aggH_ps[:])
                    nc.scalar.dma_start(aggH[b * 128:(b + 1) * 128, :], aH[:])
                s["hid2"] = hid2

            def stageC(i):
                b, c, cc = sched[i]
                aggI_ps = blk_tiles[b][1]
                s = ch.pop(i)
                nc.tensor.matmul(aggI_ps[:], s["S_c"][:], s["hid2"][:],
                                 start=(c == 0), stop=(c == CAPS[b] - 1),
                                 skip_group_check=True)
                if c == CAPS[b] - 1:
                    aI = io2.tile([128, D], BF16, tag="aI")
                    nc.vector.tensor_copy(aI[:], aggI_ps[:])
                    nc.scalar.dma_start(aggI[b * 128:(b + 1) * 128, :], aI[:])

            for i in range(NSCH + 3):
                if i < NSCH:
                    stageA(i)
                if 2 <= i <= NSCH + 1:
                    stageB(i - 2)
                if i >= 3:
                    stageC(i - 3)

            # ---- P4: node + output MLPs, 512-row blocks, staged so the
            # PE never waits on the silu of the stage it just fed
            NRB = NGS // 512
            st = {}

            def p4_load(rb):
                cnt_sb = io.tile([2, 4, 128], BF16, tag="cnt")
                nc.sync.dma_start(cnt_sb[:], cnt_v[:, rb, :, :])
                gT = io2.tile([128, 4, 4, 128], BF16, tag="gT4")
                nc.sync.dma_start(gT[:], gridT_v[:, :, rb, :, :])
                isl = iong_sb[:, rb // 4, (rb % 4) * 32:(rb % 4) * 32 + 32]
                aHT = io2.tile([128, 4, D], BF16, tag="aHT")
                nc.gpsimd.dma_gather(aHT[:], aggH[:], isl,
                                     num_idxs=512, num_idxs_reg=reg_of(512),
                                     elem_size=D, transpose=True)
                aIT = io2.tile([128, 4, D], BF16, tag="aIT")
                nc.gpsimd.dma_gather(aIT[:], aggI[:], isl,
                                     num_idxs=512, num_idxs_reg=reg_of(512),
                                     elem_size=D, transpose=True)
                st[rb] = dict(cnt=cnt_sb, gT=gT, aHT=aHT, aIT=aIT)

            def p4_h3(rb):
                s = st[rb]
                h3 = io2.tile([128, 4, 512], BF16, tag="h3")
                for g in range(4):
                    ps3 = psA.tile([128, 512], F32, tag="mm")
                    for k in range(4):
                        nc.tensor.matmul(ps3[:], w_sb["w0a"][:, k, g, :],
                                         s["gT"][:, k], start=(k == 0),
                                         stop=False)
                    for k in range(4):
                        nc.tensor.matmul(ps3[:], w_sb["u1"][:, k, g, :],
                                         s["aHT"][:, k, :], start=False,
                                         stop=False)
                    for k in range(4):
                        nc.tensor.matmul(ps3[:], w_sb["u2"][:, k, g, :],
                                         s["aIT"][:, k, :], start=False,
                                         stop=False)
                    nc.tensor.matmul(ps3[:], v3b3_sb[:, g, :], s["cnt"][:],
                                     start=False, stop=True)
                    nc.scalar.activation(h3[:, g, :], ps3[:], AF.Silu)
                s["h3"] = h3

            def p4_h4(rb):
                s = st[rb]
                h4 = io2.tile([128, 4, 512], BF16, tag="h4")
                for g in range(4):
                    ps4 = psA.tile([128, 512], F32, tag="mm")
                    for k in range(4):
                        nc.tensor.matmul(ps4[:], w_sb["ow0"][:, k, g, :],
                                         s["gT"][:, k], start=(k == 0),
                                         stop=False)
                    for k in range(4):
                        nc.tensor.matmul(ps4[:], w_sb["v"][:, k, g, :],
                                         s["h3"][:, k, :], start=False,
                                         stop=False)
                    nc.tensor.matmul(ps4[:], b4_sb[:, g, :], onesrow_sb[:],
                                     start=False, stop=True)
                    nc.scalar.activation(h4[:, g, :], ps4[:], AF.Silu)
                s["h4"] = h4

            def p4_out(rb):
                s = st.pop(rb)
                r0 = rb * 512
                for sc in range(4):
                    rs = slice(sc * 128, (sc + 1) * 128)
                    pso = psA.tile([128, 512], F32, tag="mm")
                    for k in range(4):
                        nc.tensor.matmul(pso[:], s["h4"][:, k, rs],
                                         w_sb["ow1"][:, k],
                                         start=(k == 0), stop=False)
                    nc.tensor.matmul(pso[:], ones1_sb[:], ob1_sb[:],
                                     start=False, stop=True)
                    ot = io2.tile([128, 512], BF16, tag="ot")
                    nc.vector.tensor_copy(ot[:], pso[:])
                    nc.scalar.dma_start(
                        outt[r0 + sc * 128:r0 + (sc + 1) * 128, :],
                        ot[:, :OUTD])

            for rb in range(NRB + 2):
                if rb < NRB:
                    p4_load(rb)
                    p4_h3(rb)
                if 1 <= rb <= NRB:
                    p4_h4(rb - 1)
                if rb >= 2:
                    p4_out(rb - 2)

    from concourse.library_overlay import lower_extended_insts
    lower_extended_insts(nc)   # fill .instr of InstISA subclasses (load_library)
    if SPLIT_WAITS:
        _split_multi_waits(nc)
    return nc


def _split_multi_waits(nc):
    """This walrus build allows at most ONE sync wait per instruction.
    Move surplus waits onto EventSemaphore carrier instructions inserted
    immediately before, on the same engine (semantically identical: the
    sequencer blocks on each in order)."""
    for f in nc.m.functions:
        for bb in f.blocks:
            insts = list(bb.instructions)
            if not any(i.sync_info is not None and len(i.sync_info.on_wait) > 1
                       for i in insts):
                continue
            new = []
            for ins in insts:
                si = ins.sync_info
                if si is not None and len(si.on_wait) > 1:
                    waits = list(si.on_wait)
                    for w in waits[:-1]:
                        c = mybir.InstEventSemaphore(
                            name=f"I-w{nc.next_id()}", engine=ins.engine,
                            ins=[], outs=[],
                            sync_info=mybir.SyncInfo(on_wait=[w], on_update=[]))
                        new.append(c)
                    del si.on_wait[:]
                    si.on_wait.append(waits[-1])
                new.append(ins)
            bb.instructions = new


# ------------------------------------------------------------ host pipeline
def _prep(inputs):
    """Host-side fold/sort/pack. Returns (in_maps, CAPS, perm)."""
    mesh_f = np.asarray(inputs["mesh_node_features"])[0]   # [N_MESH, D]
    grid_f = np.asarray(inputs["grid_node_features"])[0]   # [N_GRID, D]
    attrs = np.asarray(inputs["edge_attrs"])               # [E, 4]
    esrc = np.asarray(inputs["edge_src"]).astype(np.int64)
    edst = np.asarray(inputs["edge_dst"]).astype(np.int64)

    # ---- fold weights (fp32 on host, cast bf16)
    W = {k: np.asarray(inputs[k], np.float32) for k in (
        "emb_w0", "emb_b0", "emb_w1", "emb_b1", "edge_w0", "edge_b0",
        "edge_w1", "edge_b1", "node_w0", "node_b0", "node_w1", "node_b1",
        "out_w0", "out_b0", "out_w1", "out_b1")}
    Ws, Wd, We = W["edge_w0"][:D], W["edge_w0"][D:2 * D], W["edge_w0"][2 * D:]
    W0a, W0b = W["node_w0"][:D], W["node_w0"][D:]
    W_he = W["emb_w1"] @ We
    b2 = W["emb_b1"] @ We + W["edge_b0"]
    U1 = W["emb_w1"] @ W0b
    U2 = W["edge_w1"] @ W0b
    v3 = (W["emb_b1"] + W["edge_b1"]) @ W0b
    V = W["node_w1"] @ W["out_w0"]
    b4 = W["node_b1"] @ W["out_w0"] + W["out_b0"]
    emb_w0b = np.concatenate([W["emb_w0"], W["emb_b0"][None]], 0)  # [5, D]
    v3b3 = np.stack([v3, W["node_b0"]], 0)                          # [2, D]
    ow1p = np.zeros((D, 512), np.float32)
    ow1p[:, :OUTD] = W["out_w1"]
    ob1p = np.zeros(512, np.float32)
    ob1p[:OUTD] = W["out_b1"]

    # ---- sort edges by global dst block, balance blocks across cores
    order = np.argsort(edst, kind="stable")
    esrc, edst, attrs = esrc[order], edst[order], attrs[order]
    gblk = edst // 128                       # global 128-row dst block
    counts = np.bincount(gblk, minlength=NGB)
    blk_rank = np.argsort(counts)[::-1]      # descending edge count
    # slot assignment: rank r -> core r%8, slot r//8 (pad to 512 with -1)
    blocks = np.full(NCORES * NBLK, -1, np.int64)
    blocks[:NGB] = blk_rank
    core_of_rank = np.arange(NCORES * NBLK) % NCORES
    slot_of_rank = np.arange(NCORES * NBLK) // NCORES
    CAPS = np.zeros(NBLK, np.int64)
    for s in range(NBLK):
        gbs = blocks[s * NCORES:(s + 1) * NCORES]
        mx = max(int(counts[g]) if g >= 0 else 0 for g in gbs)
        CAPS[s] = max(1, _cdiv(mx, 128))
    CAPS = tuple(int(x) for x in CAPS)
    NCHUNK = sum(CAPS)
    ECP = NCHUNK * 128
    NCP = _pad128(NCHUNK)
    IW = _pad128(ECP // 16)
    chunk_off = np.concatenate([[0], np.cumsum(CAPS)])

    # per-core block table: core c slot s -> global block id (or -1)
    blk_of = np.full((NCORES, NBLK), -1, np.int64)
    for rnk in range(NCORES * NBLK):
        blk_of[core_of_rank[rnk], slot_of_rank[rnk]] = blocks[rnk]

    meshT = np.zeros((D, NM), np.float32)
    meshT[:, :N_MESH] = mesh_f.T
    iotaNG = _wrap_idx(np.arange(NGS))
    iota128 = np.tile(np.arange(128, dtype=np.float32)[None], (128, 1))
    iotacol = np.zeros((128, 128), np.float32)
    iotacol[:, 0] = np.arange(128)

    shared_parts = []

    def add(name, arr):
        shared_parts.append((name, np.asarray(arr, np.float32)))

    add("meshT", meshT.reshape(-1, 128))
    # gridT placeholder per core (inserted at same offset per core below)
    add("gridT", np.zeros((NGS * 4, 128), np.float32))
    for nm_, w_ in (("ws", Ws), ("wd", Wd), ("whe", W_he), ("u1", U1),
                    ("u2", U2), ("w0a", W0a), ("ow0", W["out_w0"]),
                    ("v", V), ("ow1", ow1p)):
        add(nm_, w_.reshape(-1, 128))
    add("emb0", emb_w0b.reshape(-1, 128))
    add("v3b3", v3b3.reshape(-1, 128))
    add("b2", b2.reshape(-1, 128))
    add("b4", b4.reshape(-1, 128))
    add("ob1", ob1p.reshape(-1, 128))
    add("iota128", iota128)
    add("ident", np.eye(128, dtype=np.float32))
    add("iotacol", iotacol)
    add("cntones", np.zeros((128, 128), np.float32))   # per-core below
    add("attrs5", np.zeros((5 * ECP // 128, 128), np.float32))
    add("dstrep", np.zeros((ECP, 128), np.float32))
    add("dstcol", np.zeros((NCP, 128), np.float32))

    names = [n for n, _ in shared_parts]
    base_arrays = {n: a for n, a in shared_parts}
    per_core_names = {"gridT", "cntones", "attrs5", "dstrep", "dstcol"}

    in_maps = []
    perm = []            # (core, local_row0, global_row0, nrows)
    for core in range(NCORES):
        src_p = np.zeros(ECP, np.int64)
        dib_p = np.full(ECP, 999.0, np.float32)
        att_p = np.zeros((ECP, 4), np.float32)
        grid_b = np.zeros((NGS, D), np.float32)
        cnt = np.zeros(NGS, np.float32)
        for s in range(NBLK):
            gb = blk_of[core, s]
            if gb < 0:
                continue
            g0 = gb * 128
            g1 = min(g0 + 128, N_GRID)
            sz = g1 - g0
            grid_b[s * 128:s * 128 + sz] = grid_f[g0:g1]
            perm.append((core, s * 128, g0, sz))
            m = gblk == gb
            n = int(m.sum())
            o = chunk_off[s] * 128
            assert n <= CAPS[s] * 128
            src_p[o:o + n] = esrc[m]
            dloc = (edst[m] - g0).astype(np.float32)
            dib_p[o:o + n] = dloc
            att_p[o:o + n] = attrs[m]
            cnt[s * 128:s * 128 + sz] = np.bincount(
                dloc.astype(np.int64), minlength=128)[:sz]

        attrsT5 = np.concatenate(
            [att_p.T, np.ones((1, ECP), np.float32)], 0)       # [5, ECP]
        dstrep = np.tile(dib_p[None], (128, 1))                 # [128, ECP]
        dstcol = np.zeros((128, NCP), np.float32)
        dstcol[:, :NCHUNK] = np.ascontiguousarray(
            dib_p.reshape(-1, 128).T)
        cntones = np.zeros((2, NGS), np.float32)
        cntones[0] = cnt
        cntones[1] = 1.0

        parts = []
        for n in names:
            if n == "gridT":
                parts.append(grid_b.T.reshape(-1, 128))
            elif n == "cntones":
                parts.append(cntones.reshape(-1, 128))
            elif n == "attrs5":
                parts.append(attrsT5.reshape(-1, 128))
            elif n == "dstrep":
                parts.append(dstrep.reshape(-1, 128))
            elif n == "dstcol":
                parts.append(dstcol.reshape(-1, 128))
            else:
                parts.append(base_arrays[n])
        blob = np.concatenate([p.astype(bf) for p in parts], 0)

        srcw = np.zeros((128, IW), np.int16)
        srcw[:, :ECP // 16] = _wrap_idx(src_p)
        idx = np.concatenate([np.ascontiguousarray(srcw).reshape(-1, 128),
                              iotaNG.reshape(-1, 128)], 0).astype(np.int16)
        in_maps.append({"blob": np.ascontiguousarray(blob),
                        "idx16": np.ascontiguousarray(idx)})
    return in_maps, CAPS, perm


_CACHE = {}


class _Runner:
    """Persistent jitted SPMD executor (avoids re-jitting per call)."""

    def __init__(self, nc):
        import jax
        from jax.experimental.shard_map import shard_map
        from jax.sharding import Mesh, PartitionSpec
        from concourse import bass2jax

        bass2jax.install_neuronx_cc_hook()
        self.nc = nc
        part_name = (nc.partition_id_tensor.name
                     if nc.partition_id_tensor else None)
        in_names, out_names, out_avals, zero_outs = [], [], [], []
        for alloc in nc.m.functions[0].allocations:
            if not isinstance(alloc, mybir.MemoryLocationSet):
                continue
            name = alloc.memorylocations[0].name
            if alloc.kind == "ExternalInput":
                if name != part_name:
                    in_names.append(name)
            elif alloc.kind == "ExternalOutput":
                shape = tuple(alloc.tensor_shape)
                dtype = mybir.dt.np(alloc.dtype)
                out_names.append(name)
                out_avals.append(jax.core.ShapedArray(shape, dtype))
                zero_outs.append(np.zeros(shape, dtype))
        self.in_names = list(in_names)
        self.out_names = out_names
        self.out_shapes = [tuple(a.shape) for a in out_avals]
        all_names = in_names + out_names
        if part_name is not None:
            all_names = all_names + [part_name]

        def _body(*args):
            operands = list(args)
            if part_name is not None:
                operands.append(bass2jax.partition_id_tensor())
            outs = bass2jax._bass_exec_p.bind(
                *operands,
                out_avals=tuple(out_avals),
                in_names=tuple(all_names),
                out_names=tuple(out_names),
                lowering_input_output_aliases=(),
                sim_require_finite=True,
                sim_require_nnan=True,
                nc=nc,
            )
            return tuple(outs)

        devices = jax.devices()[:NCORES]
        mesh = Mesh(np.asarray(devices), ("core",))
        nin = len(self.in_names) + len(out_names)
        self.fn = jax.jit(shard_map(
            _body, mesh=mesh,
            in_specs=(PartitionSpec("core"),) * nin,
            out_specs=(PartitionSpec("core"),) * len(out_names),
            check_rep=False))
        self.zero_outs = zero_outs
        self.sharding = jax.sharding.NamedSharding(mesh, PartitionSpec("core"))
        self.mesh = mesh
        self._avals = out_avals
        self._jax = jax

    def put(self, in_maps):
        """Concatenate per-core inputs on axis 0, device_put pre-sharded."""
        arrs = []
        for name in self.in_names:
            arrs.append(np.concatenate([m[name] for m in in_maps], axis=0))
        for z in self.zero_outs:
            arrs.append(np.concatenate([z] * NCORES, axis=0))
        return [self._jax.device_put(a, self.sharding) for a in arrs]

    def run(self, arrs):
        return self.fn(*arrs)

    def get(self, outs):
        res = [np.asarray(o) for o in outs]
        per_core = []
        for c in range(NCORES):
            d = {}
            for i, name in enumerate(self.out_names):
                n0 = self.out_shapes[i][0]
                d[name] = res[i][c * n0:(c + 1) * n0]
            per_core.append(d)
        return per_core


def _get_runner(CAPS) -> _Runner:
    if CAPS not in _CACHE:
        _CACHE[CAPS] = _Runner(build_bass(CAPS))
    return _CACHE[CAPS]


def kernel(**inputs) -> np.ndarray:
    in_maps, CAPS, perm = _prep(inputs)
    r = _get_runner(CAPS)
    outs = r.run(r.put(in_maps))
    per_core = r.get(outs)
    out = np.zeros((N_GRID, OUTD), np.float32)
    for core, l0, g0, sz in perm:
        o = per_core[core]["outt"]
        out[g0:g0 + sz] = o[l0:l0 + sz].astype(np.float32)
    return out[None]
